# revision 12
# baseline (speedup 1.0000x reference)
"""Trainium2 Bass kernel for a causal self-attention block with LoRA adapters.

Model (B=2, T=2048, C=1024, H=16 heads, hd=64, LoRA r=32, scale 0.5):
    qkv = x @ w_attn.T + 0.5*(x @ la_attn.T) @ lb_attn.T      (biases are 0)
    y   = causal_softmax_attention(q, k, v)
    out = y @ w_proj.T + 0.5*(y @ la_proj.T) @ lb_proj.T

Sharding: 8 cores = 2 batches x 4 head-groups. Core c owns batch c//4 and
heads 4*(c%4)..4*(c%4)+3: column-split c_attn (its 768 q/k/v rows over its
batch's 2048 tokens), full attention for its 4 heads, row-split c_proj
producing a 4-way partial [C, T]; the host sums 4 partials per batch.

Device algorithm per core (fp32 PSUM everywhere):
  - LoRA folded into effective weights on the host.  The big GEMMs use
    fp8e4m3 DoubleRow matmuls (0.5 PE rows/cycle, 256-deep contraction):
    * qkv: x and W shipped as (hi, lo) fp8 pairs (x_s = 4x, W_s = 128W);
      3-term product Whi*(xhi+xlo) + Wlo*xhi.  The hi/lo pair rides dim1 of
      one DoubleRow matmul with the other operand broadcast (0-stride), so
      the 3 terms cost 6 bf16-equivalent passes instead of 8.
    * S = q.k: k is stored as an (hi, lo) fp8 pair; one DoubleRow matmul
      per key-tile contracts [k_hi; k_lo] x [q; q] (broadcast) -> k at full
      precision, only q carries fp8 rounding.  Half the bf16 PE time.
    * causal mask: folded into the S accumulation group as one extra bf16
      matmul  st[:,diag] += A^T B  (A = strict upper ones, B = -16384*I),
      so exp() gives exact zeros and no per-block mask multiply is needed.
    * AV and c_proj stay bf16 (P cannot be quantized to fp8 cheaply).
  - attention per (j2: 1024-wide q chunk, h): S^T[k, q] blocks into PSUM,
    P = exp(S * 2^-7) on ScalarE; AV in [q, d] orientation with a 0.25
    column appended to v so yp[:,64] = den/4 and ys = yp * (4/den) = 4y.
  - normalize while tokens are on partitions (DVE reciprocal + 8 scaled
    PSUM->SBUF copies), transpose back to [ch, tok] via XBAR DMA transpose
    (mid-stream) or PE identity matmul (latency-critical tail).
  - outT_partial = Wp^T @ yn per 128-channel tile, *0.25 fused into the
    PSUM->SBUF copies.  Schedule: qkv/proj chunks drain into PE gaps in
    priority bands so neither PE nor the ScalarE exp stream starves.
Output: bf16 partial [C, T] per core; host sums 4 partials per batch in f32.
"""

from contextlib import ExitStack

import numpy as np
import ml_dtypes

import concourse.bass as bass
import concourse.tile as tile
from concourse import bacc, mybir
from concourse.bass_utils import run_bass_kernel_spmd

F32 = mybir.dt.float32
BF16 = mybir.dt.bfloat16
FP8 = mybir.dt.float8e4
AF = mybir.ActivationFunctionType
ALU = mybir.AluOpType
DR = mybir.MatmulPerfMode.DoubleRow

B, T, C, H, R = 2, 2048, 1024, 16, 32
HD = C // H              # 64
NCORES = 8
HPC = 4                  # heads per core
CH = HPC * HD            # 256 per-core channels
NCT = C // 128           # 8 contraction tiles
NQR = 3 * CH             # 768 qkv rows per core
NMT = 2 * CH // 128      # 4 q+k partition tiles
KT = T // 128            # 16 key tiles
QW = 1024                # q chunk width
TCH = 512                # token chunk for qkv/proj
NTC = T // TCH           # 4

XS = 4.0                 # host scale on x
WS = 128.0               # host scale on w_attn
QKV_SCALE = 2.0 ** -7    # psum (= 512 * raw) -> 4 * raw for q/k fp8
V_SCALE = 2.0 ** -9      # psum -> raw for v (bf16)
EXP_SCALE = 2.0 ** -7    # S_psum = 16 * S_raw; want exp(S_raw / 8)
ONES_VAL = 0.25          # v denominator column -> ys = 4 * y
PROJ_SCALE = 0.25        # proj psum (= 4 * out) -> out
MASK_VAL = -16384.0      # masked S entries (exp -> 0)

_CACHE: dict = {}
_PHASE_MARKS: list = []
_ABLATE: set = set()
_DEBUG = False


def _mark(nc, name):
    _PHASE_MARKS.append((name, nc.next_id()))


def _emit(ctx: ExitStack, tc: tile.TileContext, t_in: dict, outT, reps: int = 1):
    nc = tc.nc
    _PHASE_MARKS.clear()
    _mark(nc, "setup")

    singles = ctx.enter_context(tc.tile_pool(name="singles", bufs=1))
    psS = ctx.enter_context(tc.tile_pool(name="psS", bufs=2, space=bass.MemorySpace.PSUM))
    psY = ctx.enter_context(tc.tile_pool(name="psY", bufs=1, space=bass.MemorySpace.PSUM))
    psA = ctx.enter_context(tc.tile_pool(name="psA", bufs=2, space=bass.MemorySpace.PSUM))
    ptp = ctx.enter_context(tc.tile_pool(name="ptp", bufs=24))
    ysp = ctx.enter_context(tc.tile_pool(name="ysp", bufs=8))
    rcp = ctx.enter_context(tc.tile_pool(name="rcp", bufs=8))
    outp = ctx.enter_context(tc.tile_pool(name="outp", bufs=8))

    # ---------- constants / weights to SBUF ----------
    # x ships as fp8 (hi, lo) pairs laid out [128, ct, hl, tok] per 512-token
    # chunk; weights as fp8 hi + lo planes. Queues: scalar (ACT) carries
    # weights, sync (SP) the x head, gpsimd (Pool SWDGE) the x tail.
    xq = [singles.tile([128, NCT, 2, TCH], FP8, name=f"xq{i}")
          for i in range(NTC)]
    # hi plane duplicated in dim2: a 0-stride broadcast AP works for the
    # moving operand but NOT for the stationary (ldweights) side.
    wqh = singles.tile([128, NCT, 2, NQR], FP8)
    wql = singles.tile([128, NCT, NQR], FP8)
    wp_sb = singles.tile([128, 2, C], BF16)
    mask_sb = singles.tile([128, 2, 128], BF16)  # [:,0]=A ones, [:,1]=B diag

    _mark(nc, "xload")

    def xload(tc8, half, queue):
        queue.dma_start(
            xq[tc8][:, half * 4:(half + 1) * 4, :, :],
            t_in["xq"][:, tc8, half])

    nc.scalar.dma_start(wqh[:, 0:4], t_in["wq_hi"][:, 0:4])
    nc.scalar.dma_start(wql[:, 0:4], t_in["wq_lo"][:, 0:4])
    if "xload" not in _ABLATE:
        xload(0, 0, nc.sync)
        xload(0, 1, nc.sync)
    nc.scalar.dma_start(wqh[:, 4:8], t_in["wq_hi"][:, 4:8])
    nc.scalar.dma_start(wql[:, 4:8], t_in["wq_lo"][:, 4:8])
    nc.scalar.dma_start(mask_sb[:], t_in["masks"][:])
    nc.scalar.dma_start(wp_sb[:], t_in["wp_eff"][:])
    if "xload" not in _ABLATE:
        xload(1, 0, nc.sync)
        xload(1, 1, nc.sync)
        for tc8 in range(2, 4):
            for half in range(2):
                xload(tc8, half, nc.gpsimd)

    for _rep in range(reps):
        q8 = singles.tile([128, 2, T], FP8)        # q as fp8, 4*q_raw
        khl = singles.tile([128, 2, 2, T], FP8)    # k (hi, lo) pairs
        v1 = singles.tile([128, HPC, KT, HD + 1], BF16)
        nc.vector.memset(v1[:, :, :, HD:HD + 1], ONES_VAL)
        yn = singles.tile([128, 2, T], BF16)       # yn.T = 4*y per ch tile
        if "attn" in _ABLATE:
            nc.vector.memset(yn[:], 1.0)

        def qkv_matmuls(ps, tc8, rows, nw, tok=slice(0, TCH),
                        x_station=False):
            """12 DoubleRow matmuls: Whi*(xhi+xlo) + Wlo*xhi."""
            wh, wl = wqh[:, :, :, rows], wql[:, :, rows]
            for c in range(NCT):
                lhsT = wh[:, c]
                rhs = xq[tc8][:, c, :, tok]
                if x_station:
                    lhsT, rhs = rhs, lhsT
                nc.tensor.matmul(ps[:], lhsT, rhs, start=(c == 0),
                                 stop=False, perf_mode=DR)
            for cp in range(NCT // 2):
                lhsT = wl[:, 2 * cp:2 * cp + 2]
                rhs = xq[tc8][:, 2 * cp:2 * cp + 2, 0, tok]
                if x_station:
                    lhsT, rhs = rhs, lhsT
                nc.tensor.matmul(ps[:], lhsT, rhs, start=False,
                                 stop=(cp == NCT // 2 - 1), perf_mode=DR)

        def emit_qk_chunk(tc8, mt, eng="dve"):
            sl = slice(tc8 * TCH, (tc8 + 1) * TCH)
            ps = psA.tile([128, TCH], F32, tag="a", name=f"qk{tc8}_{mt}")
            qkv_matmuls(ps, tc8, slice(mt * 128, (mt + 1) * 128), 128)
            if mt < 2:  # q -> single fp8
                if eng == "act":
                    nc.scalar.activation(q8[:, mt, sl], ps[:], AF.Copy,
                                         scale=QKV_SCALE)
                else:
                    nc.vector.tensor_scalar(q8[:, mt, sl], ps[:],
                                            QKV_SCALE, None, ALU.mult)
            else:       # k -> (hi, lo) fp8 pair
                kh = khl[:, mt - 2, 0, sl]
                if eng == "act":
                    nc.scalar.activation(kh, ps[:], AF.Copy, scale=QKV_SCALE)
                else:
                    nc.vector.tensor_scalar(kh, ps[:], QKV_SCALE, None,
                                            ALU.mult)
                nc.vector.scalar_tensor_tensor(
                    khl[:, mt - 2, 1, sl], ps[:], QKV_SCALE, kh,
                    ALU.mult, ALU.subtract)

        def emit_v_chunk(kt):
            ps = psA.tile([128, CH], F32, tag="a", name=f"v{kt}",
                          padded_shape=[128, 512])
            qkv_matmuls(ps, kt // 4, slice(2 * CH, 3 * CH), CH,
                        tok=slice((kt % 4) * 128, (kt % 4 + 1) * 128),
                        x_station=True)
            nc.vector.tensor_scalar(
                v1[:, :, kt, 0:HD],
                ps[:].rearrange("p (h d) -> p h d", h=HPC),
                V_SCALE, None, ALU.mult)

        def emit_proj_single(mt, tc8, eng="dve", dmaq="sync", pool=None):
            sl = slice(tc8 * TCH, (tc8 + 1) * TCH)
            po = (pool or psA).tile([128, TCH], F32,
                                    tag="a" if pool is None else "st",
                                    name=f"po{mt}_{tc8}")
            for cht in range(2):
                nc.tensor.matmul(po[:],
                                 wp_sb[:, cht, mt * 128:(mt + 1) * 128],
                                 yn[:, cht, sl], start=(cht == 0),
                                 stop=(cht == 1))
            ot = outp.tile([128, TCH], BF16, tag="ots")
            if eng == "act":
                nc.scalar.activation(ot[:], po[:], AF.Copy, scale=PROJ_SCALE)
            else:
                nc.vector.tensor_scalar(ot[:], po[:], PROJ_SCALE, None,
                                        ALU.mult)
            getattr(nc, dmaq).dma_start(outT[mt * 128:(mt + 1) * 128, sl],
                                        ot[:])

        def emit_proj_pair(mt, pair, engs=("dve", "dve"), dmaq="gpsimd"):
            ot = outp.tile([128, 2, TCH], BF16, tag="ot")
            for half in range(2):
                tc8 = pair * 2 + half
                sl = slice(tc8 * TCH, (tc8 + 1) * TCH)
                po = psA.tile([128, TCH], F32, tag="a", name=f"po{mt}_{tc8}")
                for cht in range(2):
                    nc.tensor.matmul(po[:],
                                     wp_sb[:, cht, mt * 128:(mt + 1) * 128],
                                     yn[:, cht, sl], start=(cht == 0),
                                     stop=(cht == 1))
                if engs[half] == "act":
                    nc.scalar.activation(ot[:, half], po[:], AF.Copy,
                                         scale=PROJ_SCALE)
                else:
                    nc.vector.tensor_scalar(ot[:, half], po[:], PROJ_SCALE,
                                            None, ALU.mult)
            getattr(nc, dmaq).dma_start(
                outT[mt * 128:(mt + 1) * 128,
                     pair * 2 * TCH:(pair * 2 + 2) * TCH], ot[:])

        fillers: list = []

        def drain(n):
            # qkv fillers gate future exps: keep them at normal priority.
            # proj fillers are pure sinks: push them to low priority.
            save = tc.cur_priority
            try:
                for _ in range(min(n, len(fillers))):
                    kind, fn = fillers.pop(0)
                    tc.cur_priority = save + {"gate": 8000, "v": 12000,
                                              "sink": 16000}[kind]
                    fn()
            finally:
                tc.cur_priority = save
            return

        ys_tiles: dict = {}

        def emit_attn_head(j2, h, fill_every=2, fill_at=None,
                           split_exp=False):
            kmt = h // 2
            qmt = h // 2
            nkt = 8 * j2 + 8
            q0 = j2 * QW
            yp = psY.tile([128, 8, 128], F32, tag="yp", name=f"yp{j2}_{h}")
            rc = rcp.tile([128, 8], F32, tag="rc", name=f"rc{j2}_{h}")
            p0 = (h % 2) * 64
            if h % 2 == 0:
                ysc = ysp.tile([128, 8, 128], BF16, tag="ys",
                               name=f"ys{j2}_{h // 2}")
                ys_tiles[(j2, h // 2)] = ysc
            else:
                ysc = ys_tiles[(j2, h // 2)]
            ys = ysc[:, :, p0:p0 + HD]
            for kt in range(nkt):
                lead = (kt // 8 == j2)
                cs = 128 * (kt % 8) if lead else 0
                kp = (h % 2) * 64
                k_lhs = khl[kp:kp + 64, kmt, :, kt * 128:(kt + 1) * 128]
                st = psS.tile([128, QW], F32, tag="st", name=f"st{j2}_{h}_{kt}")
                ranges = ((cs, 512), (512, QW)) if cs < 512 else ((cs, QW),)
                for lo, hi in ranges:
                    # each range opens its own PSUM bank group (start=True);
                    # a start=False matmul here would accumulate onto stale
                    # bank contents from the previous st tile use.
                    q_rhs = q8[kp:kp + 64, qmt, q0 + lo:q0 + hi] \
                        .unsqueeze(1).broadcast_to([64, 2, hi - lo])
                    nc.tensor.matmul(st[:, lo:hi], k_lhs, q_rhs,
                                     start=True, stop=True, perf_mode=DR)
                if lead:
                    # causal mask: st[:, diag] += A^T B (-16384 above diag)
                    nc.tensor.matmul(st[:, cs:cs + 128], mask_sb[:, 0, :],
                                     mask_sb[:, 1, :], start=False, stop=True,
                                     skip_group_check=True)
                pt = ptp.tile([128, QW], BF16, tag="pt")
                if split_exp and cs < 512:
                    # halve the first unit's exps so the stream starts as
                    # soon as the first q8 chunk lands
                    nc.scalar.activation(pt[:, cs:512], st[:, cs:512],
                                         AF.Exp, scale=EXP_SCALE)
                    nc.scalar.activation(pt[:, 512:], st[:, 512:], AF.Exp,
                                         scale=EXP_SCALE)
                else:
                    nc.scalar.activation(pt[:, cs:], st[:, cs:], AF.Exp,
                                         scale=EXP_SCALE)
                # PSUM zero regions are bank-wide (2KB): only one accumulation
                # group per bank. Open each bank once (j=0/j=4 at kt=0); the
                # bank-wide pending-zero gives the other subtiles their
                # initial zeroing; close with the bank's last accumulation.
                j0 = max(0, kt - 8 * j2)
                for j in range(j0, 8):
                    nc.tensor.matmul(yp[:, j, 0:HD + 1],
                                     pt[:, j * 128:(j + 1) * 128],
                                     v1[:, h, kt, :],
                                     start=(kt == 0 and j % 4 == 0),
                                     stop=(j % 4 == 3 and kt == 8 * j2 + j))
                if kt == 8 * j2 + 3:
                    # bank 0 (subtiles 0-3) just closed: normalize its half
                    # now, 4 k-tiles before the unit ends
                    nc.vector.reciprocal(rc[:, 0:4], yp[:, 0:4, HD])
                    for j in range(4):
                        nc.vector.tensor_scalar(ys[:, j, :], yp[:, j, 0:HD],
                                                rc[:, j:j + 1], None, ALU.mult)
                if fill_at is not None:
                    if kt in fill_at:
                        drain(1)
                elif (kt + 1) % fill_every == 0:
                    drain(1)
            # bank 1 half (the last head's copies split across DVE/ACT to
            # shorten the post-stream tail)
            nc.vector.reciprocal(rc[:, 4:8], yp[:, 4:8, HD])
            tail_head = (j2 == 1 and h == HPC - 1)
            for j in range(4, 8):
                if tail_head and j % 2 == 1:
                    nc.scalar.activation(ys[:, j, :], yp[:, j, 0:HD],
                                         AF.Copy, scale=rc[:, j:j + 1])
                else:
                    nc.vector.tensor_scalar(ys[:, j, :], yp[:, j, 0:HD],
                                            rc[:, j:j + 1], None, ALU.mult)
            if _DEBUG and h == 0:
                nc.sync.dma_start(t_in["ys_dbg"][:, j2], ys[:])
                nc.sync.dma_start(t_in["rc_dbg"][:, j2], rc[:])

        def emit_dphase_half(j2, cht, half, ysc, eng="dve"):
            # transpose back: yn[ch, tok] = ys[q, ch].T
            # mid-stream phases use the XBAR DMA transpose on the idle SP
            # queue (frees PE + DVE); eng="pe" keeps the identity-matmul path
            # for the latency-critical final phase.
            if eng == "dma":
                for jj in range(4):
                    j = half * 4 + jj
                    t0 = j2 * QW + j * 128
                    nc.sync.dma_start(yn[:, cht, t0:t0 + 128],
                                      ysc[:, j, :], transpose=True)
                return
            ys_pair = [ysc[:, :, 0:HD], ysc[:, :, HD:2 * HD]]
            if True:
                dout = psA.tile([128, 512], F32, tag="a",
                                name=f"do{j2}_{cht}_{half}")
                for hh in range(2):
                    for jj in range(4):
                        j = half * 4 + jj
                        nc.tensor.matmul(dout[hh * 64:(hh + 1) * 64,
                                              jj * 128:(jj + 1) * 128],
                                         ys_pair[hh][:, j, :],
                                         mask_sb[:, 1, :],
                                         start=True, stop=True)
                t0 = j2 * QW + half * 512
                if eng == "act":
                    nc.scalar.activation(yn[:, cht, t0:t0 + 512], dout[:],
                                         AF.Copy, scale=1.0 / MASK_VAL)
                elif eng == "split":
                    nc.vector.tensor_scalar(yn[:, cht, t0:t0 + 256],
                                            dout[:, 0:256], 1.0 / MASK_VAL,
                                            None, ALU.mult)
                    nc.scalar.activation(yn[:, cht, t0 + 256:t0 + 512],
                                         dout[:, 256:512], AF.Copy,
                                         scale=1.0 / MASK_VAL)
                else:
                    nc.vector.tensor_scalar(yn[:, cht, t0:t0 + 512],
                                            dout[:], 1.0 / MASK_VAL, None,
                                            ALU.mult)

        def emit_dphase(j2, cht, engs=("dve", "dve")):
            ysc = ys_tiles.pop((j2, cht))
            for half in range(2):
                emit_dphase_half(j2, cht, half, ysc, engs[half])

        # ---------- schedule ----------
        _mark(nc, "qkv0")
        for tc8 in range(2):
            for mt in (0, 2):   # heads 0/1 q+k; ACT is idle before attention
                emit_qk_chunk(tc8, mt, eng="act")
        save_p = tc.cur_priority
        tc.cur_priority = save_p + 12000
        for kt in range(4):
            emit_v_chunk(kt)
        tc.cur_priority = save_p

        if "attn" not in _ABLATE:
            # Interleave ACT-light (j2=0) and ACT-heavy (j2=1) units so the
            # exp stream never starves regionally; fillers sized per unit.
            def qkf(tc8, mt):
                fillers.append(("gate", lambda: emit_qk_chunk(tc8, mt)))

            def vf(kt):
                fillers.append(("v", lambda: emit_v_chunk(kt)))

            _mark(nc, "attn0")
            qkf(2, 0); qkf(3, 0)
            for kt in range(4, 8):
                vf(kt)
            qkf(0, 1); qkf(0, 3)
            with tc.high_priority(offset=4000):
                emit_attn_head(0, 0, fill_at=set(range(8)), split_exp=True)
            qkf(2, 2); qkf(3, 2); qkf(1, 1); qkf(1, 3)
            with tc.high_priority(offset=4000):
                emit_attn_head(0, 1, fill_at={0, 1, 2, 3}, split_exp=True)
            emit_dphase(0, 0, engs=("dma", "dma"))
            for kt in range(8, 16):
                vf(kt)
            with tc.high_priority(offset=4000):
                emit_attn_head(1, 0, fill_at=set(range(8)))
            qkf(2, 1); qkf(3, 1); qkf(2, 3); qkf(3, 3)
            with tc.high_priority(offset=4000):
                emit_attn_head(0, 2, fill_at={1, 3, 5, 7})
            with tc.high_priority(offset=4000):
                emit_attn_head(1, 1, fill_at={3, 7, 11, 15})
            emit_dphase(1, 0, engs=("dma", "dma"))
            with tc.high_priority(offset=4000):
                emit_attn_head(0, 3, fill_at={1, 3, 5, 7})
            _mark(nc, "dphase0")
            drain(len(fillers))
            emit_dphase(0, 1, engs=("dma", "dma"))
            _mark(nc, "attn1")
            for mt in range(NCT - 2):
                fillers.append(("sink", lambda mt=mt: emit_proj_pair(mt, 0)))
            with tc.high_priority(offset=4000):
                emit_attn_head(1, 2, fill_at={1, 5, 9, 13})
            with tc.high_priority(offset=4000):
                emit_attn_head(1, 3, fill_at={1, 5, 9, 13})
            _mark(nc, "dphase1")
            for mt in (NCT - 2, NCT - 1):
                emit_proj_pair(mt, 0)
            drain(len(fillers))
            ysc_t = ys_tiles.pop((1, 1))
            with tc.high_priority(offset=4000):
                emit_dphase_half(1, 1, 0, ysc_t, "dve")
            for i, mt in enumerate(range(NCT)):
                emit_proj_single(mt, 2, eng=("act", "dve")[i % 2],
                                 dmaq=("sync", "gpsimd")[i % 2],
                                 pool=(None, psS)[i % 2])
            with tc.high_priority(offset=4000):
                emit_dphase_half(1, 1, 1, ysc_t, "act")
            for i, mt in enumerate(range(NCT)):
                emit_proj_single(mt, 3, eng=("dve", "act")[i % 2],
                                 dmaq=("gpsimd", "sync")[i % 2],
                                 pool=(None, psS)[i % 2])
        else:
            for tc8 in range(2):
                for mt in (1, 3):
                    emit_qk_chunk(tc8, mt)
            for tc8 in range(2, 4):
                for mt in range(NMT):
                    emit_qk_chunk(tc8, mt)
            for kt in range(8, 16):
                emit_v_chunk(kt)
            for mt in range(NCT):
                emit_proj_pair(mt, 0)

        _mark(nc, "projtail")
        if "proj" not in _ABLATE and "attn" in _ABLATE:
            engs = [("dve", "act"), ("act", "dve")]
            for mt in range(NCT):
                emit_proj_pair(mt, 1, engs=engs[mt % 2], dmaq="sync")

        if _DEBUG:
            nc.sync.dma_start(t_in["q8_dbg"][:], q8[:])
            nc.sync.dma_start(t_in["khl_dbg"][:], khl[:])
            nc.sync.dma_start(t_in["v1_dbg"][:], v1[:])
            nc.sync.dma_start(t_in["yn_dbg"][:], yn[:])


def _declare_io(nc):
    t_in = {
        # [128, tc8, half, (ct 4, hl 2, tok 512)] fp8
        "xq": nc.dram_tensor("xq", [128, NTC, 2, 4, 2, TCH], FP8,
                             kind="ExternalInput"),
        "wq_hi": nc.dram_tensor("wq_hi", [128, NCT, 2, NQR], FP8,
                                kind="ExternalInput"),
        "wq_lo": nc.dram_tensor("wq_lo", [128, NCT, NQR], FP8,
                                kind="ExternalInput"),
        "wp_eff": nc.dram_tensor("wp_eff", [128, 2, C], BF16,
                                 kind="ExternalInput"),
        "masks": nc.dram_tensor("masks", [128, 2, 128], BF16,
                                kind="ExternalInput"),
    }
    outT = nc.dram_tensor("outT", [C, T], BF16, kind="ExternalOutput")
    if _DEBUG:
        t_in["q8_dbg"] = nc.dram_tensor("q8_dbg", [128, 2, T], FP8,
                                        kind="ExternalOutput")
        t_in["khl_dbg"] = nc.dram_tensor("khl_dbg", [128, 2, 2, T], FP8,
                                         kind="ExternalOutput")
        t_in["v1_dbg"] = nc.dram_tensor("v1_dbg", [128, HPC, KT, HD + 1],
                                        BF16, kind="ExternalOutput")
        t_in["yn_dbg"] = nc.dram_tensor("yn_dbg", [128, 2, T], BF16,
                                        kind="ExternalOutput")
        t_in["ys_dbg"] = nc.dram_tensor("ys_dbg", [128, 2, 8, HD], BF16,
                                        kind="ExternalOutput")
        t_in["rc_dbg"] = nc.dram_tensor("rc_dbg", [128, 2, 8], F32,
                                        kind="ExternalOutput")
    return t_in, outT


def _build(reps: int = 1):
    nc = bacc.Bacc("TRN2", target_bir_lowering=False, debug=False)
    t_in, outT = _declare_io(nc)
    with tile.TileContext(nc) as tc:
        with ExitStack() as ctx:
            _emit(ctx, tc, t_in, outT, reps=reps)
    nc.compile()
    return nc


def _fp8_split(a: np.ndarray):
    """Return (hi, lo) fp8e4m3 pair with hi + lo ~= a."""
    f8 = ml_dtypes.float8_e4m3
    hi = a.astype(f8)
    lo = (a - hi.astype(np.float32)).astype(f8)
    return hi, lo


def _make_in_maps(inputs: dict) -> list:
    f32 = np.float32
    f8 = ml_dtypes.float8_e4m3
    x = np.asarray(inputs["x"], f32)                     # [B, T, C]
    w_attn = np.asarray(inputs["w_attn"], f32)
    la_attn = np.ascontiguousarray(np.asarray(inputs["la_attn"], f32))
    lb_attn = np.asarray(inputs["lb_attn"], f32)
    w_proj = np.asarray(inputs["w_proj"], f32)
    la_proj = np.asarray(inputs["la_proj"], f32)
    lb_proj = np.asarray(inputs["lb_proj"], f32)

    # fold LoRA into effective weights on the host (input preprocessing)
    Wq = w_attn + 0.5 * lb_attn @ la_attn                # [3C, C]
    Wp = w_proj + 0.5 * lb_proj @ la_proj                # [C, C]

    # x: [128, tc8, half, ct(4), hl(2), tok(512)] fp8, scaled by XS
    xq_b = []
    for b in range(B):
        xT = np.ascontiguousarray(x[b].T) * XS           # [C, T]
        # [ct8, 128, tok] -> hi/lo
        xr = xT.reshape(NCT, 128, T)
        hi, lo = _fp8_split(xr)
        arr = np.empty((128, NTC, 2, 4, 2, TCH), f8)
        for tc8 in range(NTC):
            for half in range(2):
                cs = half * 4
                blk_h = hi[cs:cs + 4, :, tc8 * TCH:(tc8 + 1) * TCH]
                blk_l = lo[cs:cs + 4, :, tc8 * TCH:(tc8 + 1) * TCH]
                arr[:, tc8, half, :, 0, :] = blk_h.transpose(1, 0, 2)
                arr[:, tc8, half, :, 1, :] = blk_l.transpose(1, 0, 2)
        xq_b.append(arr)

    k_idx = np.arange(128)[:, None]
    q_idx = np.arange(128)[None, :]
    masks = np.zeros((128, 2, 128), ml_dtypes.bfloat16)
    masks[:, 0, :] = (q_idx > k_idx)         # A[j, k] = 1 iff k > j
    masks[:, 1, :] = MASK_VAL * (q_idx == k_idx)

    in_maps = []
    for core in range(NCORES):
        b, g = core // 4, core % 4
        ch0 = g * CH
        rows = np.r_[ch0:ch0 + CH, C + ch0:C + ch0 + CH,
                     2 * C + ch0:2 * C + ch0 + CH]
        # [p, ct, r] = (WS * Wq).T[ct*128+p, r] over this core's 768 rows
        wq_s = np.ascontiguousarray(
            (WS * Wq[rows]).T.reshape(NCT, 128, NQR).transpose(1, 0, 2))
        wq_hi1, wq_lo = _fp8_split(wq_s)
        wq_hi = np.repeat(wq_hi1[:, :, None, :], 2, axis=2)
        # [p, cht, c] = Wp.T[ch0+cht*128+p, c]
        wp_eff = np.ascontiguousarray(
            Wp[:, ch0:ch0 + CH].T.reshape(2, 128, C).transpose(1, 0, 2)
        ).astype(ml_dtypes.bfloat16)
        in_maps.append({
            "xq": xq_b[b],
            "wq_hi": wq_hi,
            "wq_lo": wq_lo,
            "wp_eff": wp_eff,
            "masks": masks,
        })
    return in_maps


def _execute(inputs: dict, trace: bool = False):
    if "nc" not in _CACHE:
        _CACHE["nc"] = _build()
    nc = _CACHE["nc"]
    in_maps = _make_in_maps(inputs)
    res = run_bass_kernel_spmd(nc, in_maps, core_ids=list(range(NCORES)),
                               trace=trace)
    out = np.empty((B, T, C), np.float32)
    for b in range(B):
        acc = np.zeros((C, T), np.float32)
        for g in range(4):
            acc += np.asarray(res.results[b * 4 + g]["outT"], dtype=np.float32)
        out[b] = acc.T
    return out, res


def kernel(**inputs) -> np.ndarray:
    out, _ = _execute(inputs, trace=False)
    return out


# revision 29
# speedup vs baseline: 1.0288x; 1.0288x over previous
"""Trainium2 Bass kernel for a causal self-attention block with LoRA adapters.

Model (B=2, T=2048, C=1024, H=16 heads, hd=64, LoRA r=32, scale 0.5):
    qkv = x @ w_attn.T + 0.5*(x @ la_attn.T) @ lb_attn.T      (biases are 0)
    y   = causal_softmax_attention(q, k, v)
    out = y @ w_proj.T + 0.5*(y @ la_proj.T) @ lb_proj.T

Sharding: 8 cores = 2 batches x 4 head-groups. Core c owns batch c//4 and
heads 4*(c%4)..4*(c%4)+3: column-split c_attn (its 768 q/k/v rows over its
batch's 2048 tokens), full attention for its 4 heads, row-split c_proj
producing a 4-way partial [C, T]; the host sums 4 partials per batch.

Device algorithm per core (fp32 PSUM everywhere):
  - LoRA folded into effective weights on the host.  The big GEMMs use
    fp8e4m3 DoubleRow matmuls (0.5 PE rows/cycle, 256-deep contraction):
    * qkv: x and W shipped as (hi, lo) fp8 pairs (x_s = 4x, W_s = 128W);
      3-term product Whi*(xhi+xlo) + Wlo*xhi.  The hi/lo pair rides dim1 of
      one DoubleRow matmul with the other operand broadcast (0-stride), so
      the 3 terms cost 6 bf16-equivalent passes instead of 8.
    * S = q.k: k is stored as an (hi, lo) fp8 pair; one DoubleRow matmul
      per key-tile contracts [k_hi; k_lo] x [q; q] (broadcast) -> k at full
      precision, only q carries fp8 rounding.  Half the bf16 PE time.
    * causal mask: folded into the S accumulation group as one extra bf16
      matmul  st[:,diag] += A^T B  (A = strict upper ones, B = -16384*I),
      so exp() gives exact zeros and no per-block mask multiply is needed.
    * AV and c_proj stay bf16 (P cannot be quantized to fp8 cheaply).
  - attention per (j2: 1024-wide q chunk, h): S^T[k, q] blocks into PSUM,
    P = exp(S * 2^-7) on ScalarE; AV in [q, d] orientation with a 0.25
    column appended to v so yp[:,64] = den/4 and ys = yp * (4/den) = 4y.
  - normalize while tokens are on partitions (DVE reciprocal + 8 scaled
    PSUM->SBUF copies), transpose back to [ch, tok] via XBAR DMA transpose
    (mid-stream) or PE identity matmul (latency-critical tail).
  - outT_partial = Wp^T @ yn per 128-channel tile, *0.25 fused into the
    PSUM->SBUF copies.  Schedule: qkv/proj chunks drain into PE gaps in
    priority bands so neither PE nor the ScalarE exp stream starves.
Output: bf16 partial [C, T] per core; host sums 4 partials per batch in f32.
"""

from contextlib import ExitStack

import numpy as np
import ml_dtypes

import concourse.bass as bass
import concourse.tile as tile
from concourse import bacc, mybir
from concourse.bass_utils import run_bass_kernel_spmd

F32 = mybir.dt.float32
BF16 = mybir.dt.bfloat16
FP8 = mybir.dt.float8e4
AF = mybir.ActivationFunctionType
ALU = mybir.AluOpType
DR = mybir.MatmulPerfMode.DoubleRow

B, T, C, H, R = 2, 2048, 1024, 16, 32
HD = C // H              # 64
NCORES = 8
HPC = 4                  # heads per core
CH = HPC * HD            # 256 per-core channels
NCT = C // 128           # 8 contraction tiles
NQR = 3 * CH             # 768 qkv rows per core
NMT = 2 * CH // 128      # 4 q+k partition tiles
KT = T // 128            # 16 key tiles
QW = 1024                # q chunk width
TCH = 512                # token chunk for qkv/proj
NTC = T // TCH           # 4

XS = 4.0                 # host scale on x
WS = 128.0               # host scale on w_attn
QKV_SCALE = 2.0 ** -7    # psum (= 512 * raw) -> 4 * raw for q/k fp8
V_SCALE = 2.0 ** -9      # psum -> raw for v (bf16)
EXP_SCALE = 2.0 ** -7    # S_psum = 16 * S_raw; want exp(S_raw / 8)
ONES_VAL = 0.25          # v denominator column -> ys = 4 * y
PROJ_SCALE = 0.25        # proj psum (= 4 * out) -> out
MASK_VAL = -16384.0      # masked S entries (exp -> 0)

_CACHE: dict = {}
_PHASE_MARKS: list = []
_ABLATE: set = set()
_DEBUG = False


def _mark(nc, name):
    _PHASE_MARKS.append((name, nc.next_id()))


def _emit(ctx: ExitStack, tc: tile.TileContext, t_in: dict, outT, reps: int = 1):
    nc = tc.nc
    _PHASE_MARKS.clear()
    _mark(nc, "setup")

    singles = ctx.enter_context(tc.tile_pool(name="singles", bufs=1))
    psS = ctx.enter_context(tc.tile_pool(name="psS", bufs=2, space=bass.MemorySpace.PSUM))
    psY = ctx.enter_context(tc.tile_pool(name="psY", bufs=1, space=bass.MemorySpace.PSUM))
    psA = ctx.enter_context(tc.tile_pool(name="psA", bufs=2, space=bass.MemorySpace.PSUM))
    ptp = ctx.enter_context(tc.tile_pool(name="ptp", bufs=24))
    ysp = ctx.enter_context(tc.tile_pool(name="ysp", bufs=8))
    rcp = ctx.enter_context(tc.tile_pool(name="rcp", bufs=8))
    outp = ctx.enter_context(tc.tile_pool(name="outp", bufs=8))

    # ---------- constants / weights to SBUF ----------
    # x ships as fp8 (hi, lo) pairs laid out [128, ct, hl, tok] per 512-token
    # chunk; weights as fp8 hi + lo planes. Queues: scalar (ACT) carries
    # weights, sync (SP) the x head, gpsimd (Pool SWDGE) the x tail.
    # x per 512-token chunk: [128, ct, hl, tok] fp8, with the hi/lo planes
    # shipped as separate DMAs (512B descriptor runs) so the T1+T3 matmuls
    # (hi-only) can start before the lo planes land.
    xq = [singles.tile([128, NCT, 2, TCH], FP8, name=f"xq{i}")
          for i in range(NTC)]
    # weights per row-block, (hi, lo) interleaved on dim2: one DoubleRow
    # matmul per c computes Whi*xhi + Wlo*xhi with the pair as stationary;
    # the Whi*xlo correction pairs c-planes (stride-2 APs into the same
    # tiles).
    wq_sb = singles.tile([128, NCT, 2, 256], FP8)  # q rows 0:256
    wk_sb = singles.tile([128, NCT, 2, 256], FP8)  # k rows 256:512
    wv_sb = singles.tile([128, NCT, 2, 256], FP8)  # v rows 512:768
    wp_sb = singles.tile([128, 2, C], BF16)
    mask_sb = singles.tile([128, 2, 128], BF16)  # [:,0]=A ones, [:,1]=B diag

    _mark(nc, "xload")

    def xload(tc8, hl, queue, crange=slice(0, NCT)):
        queue.dma_start(
            xq[tc8][:, crange, hl:hl + 1, :],
            t_in["xq"][:, tc8, hl, crange].unsqueeze(2))

    nc.scalar.dma_start(mask_sb[:], t_in["masks"][:])
    nc.scalar.dma_start(wk_sb[:], t_in["w_k"][:])
    if "xload" not in _ABLATE:
        xload(0, 0, nc.sync, slice(0, 4))
        xload(0, 0, nc.sync, slice(4, 8))
    nc.scalar.dma_start(wq_sb[:], t_in["w_q"][:])
    if "xload" not in _ABLATE:
        xload(0, 1, nc.scalar, slice(0, 4))
        xload(0, 1, nc.scalar, slice(4, 8))
        xload(1, 0, nc.sync)
    nc.scalar.dma_start(wv_sb[:], t_in["w_v"][:])
    if "xload" not in _ABLATE:
        xload(1, 1, nc.sync)
    nc.scalar.dma_start(wp_sb[:], t_in["wp_eff"][:])
    if "xload" not in _ABLATE:
        for tc8 in range(2, 4):
            for hl in range(2):
                xload(tc8, hl, nc.gpsimd)

    for _rep in range(reps):
        q8 = singles.tile([128, 2, T], FP8)        # q as fp8, 4*q_raw
        khl = singles.tile([128, 2, 2, T], FP8)    # k (hi, lo) pairs
        v1 = singles.tile([128, HPC, KT, HD + 1], BF16)
        nc.vector.memset(v1[:, :, :, HD:HD + 1], ONES_VAL)
        yn = singles.tile([128, 2, T], BF16)       # yn.T = 4*y per ch tile
        if "attn" in _ABLATE:
            nc.vector.memset(yn[:], 1.0)

        def qkv_matmuls(ps, tc8, wt, rows, nw, tok=slice(0, TCH),
                        x_station=False):
            """12 DoubleRow matmuls: (Whi+Wlo)*xhi + Whi*xlo."""
            for c in range(NCT):
                # T1+T3: stationary (Whi[c], Wlo[c]) pair x broadcast xhi[c]
                # (x-stationary flavor: (xhi[c], xlo[c]) x broadcast Whi[c])
                if x_station:
                    lhsT = xq[tc8][:, c, :, tok]
                    rhs = wt[:, c, 0, rows].unsqueeze(1) \
                        .broadcast_to([128, 2, nw])
                else:
                    lhsT = wt[:, c, :, rows]
                    rhs = xq[tc8][:, c, 0, tok].unsqueeze(1) \
                        .broadcast_to([128, 2, TCH])
                nc.tensor.matmul(ps[:], lhsT, rhs, start=(c == 0),
                                 stop=False, perf_mode=DR)
            for cp in range(NCT // 2):
                # T2: (Whi[c], Whi[c+1]) pair x (xlo[c], xlo[c+1]) pair
                # (x-stationary flavor: (xhi[c], xhi[c+1]) x (Wlo[c], Wlo[c+1]))
                if x_station:
                    lhsT = xq[tc8][:, 2 * cp:2 * cp + 2, 0, tok]
                    rhs = wt[:, 2 * cp:2 * cp + 2, 1, rows]
                else:
                    lhsT = wt[:, 2 * cp:2 * cp + 2, 0, rows]
                    rhs = xq[tc8][:, 2 * cp:2 * cp + 2, 1, tok]
                nc.tensor.matmul(ps[:], lhsT, rhs, start=False,
                                 stop=(cp == NCT // 2 - 1), perf_mode=DR)

        def emit_qk_chunk(tc8, mt, eng="dve"):
            sl = slice(tc8 * TCH, (tc8 + 1) * TCH)
            ps = psA.tile([128, TCH], F32, tag="a", name=f"qk{tc8}_{mt}")
            qkv_matmuls(ps, tc8, wq_sb if mt < 2 else wk_sb,
                        slice((mt % 2) * 128, (mt % 2 + 1) * 128), 128)
            if mt < 2:  # q -> single fp8
                if eng == "act":
                    nc.scalar.activation(q8[:, mt, sl], ps[:], AF.Copy,
                                         scale=QKV_SCALE)
                else:
                    nc.vector.tensor_scalar(q8[:, mt, sl], ps[:],
                                            QKV_SCALE, None, ALU.mult)
            else:       # k -> (hi, lo) fp8 pair
                kh = khl[:, mt - 2, 0, sl]
                if eng == "act":
                    nc.scalar.activation(kh, ps[:], AF.Copy, scale=QKV_SCALE)
                else:
                    nc.vector.tensor_scalar(kh, ps[:], QKV_SCALE, None,
                                            ALU.mult)
                nc.vector.scalar_tensor_tensor(
                    khl[:, mt - 2, 1, sl], ps[:], QKV_SCALE, kh,
                    ALU.mult, ALU.subtract)

        def emit_v_chunk(kt):
            ps = psA.tile([128, CH], F32, tag="a", name=f"v{kt}",
                          padded_shape=[128, 512])
            qkv_matmuls(ps, kt // 4, wv_sb, slice(0, CH), CH,
                        tok=slice((kt % 4) * 128, (kt % 4 + 1) * 128),
                        x_station=True)
            nc.vector.tensor_scalar(
                v1[:, :, kt, 0:HD],
                ps[:].rearrange("p (h d) -> p h d", h=HPC),
                V_SCALE, None, ALU.mult)

        def emit_proj_single(mt, tc8, eng="dve", dmaq="sync", pool=None):
            sl = slice(tc8 * TCH, (tc8 + 1) * TCH)
            po = (pool or psA).tile([128, TCH], F32,
                                    tag="a" if pool is None else "st",
                                    name=f"po{mt}_{tc8}")
            for cht in range(2):
                nc.tensor.matmul(po[:],
                                 wp_sb[:, cht, mt * 128:(mt + 1) * 128],
                                 yn[:, cht, sl], start=(cht == 0),
                                 stop=(cht == 1))
            ot = outp.tile([128, TCH], BF16, tag="ots")
            if eng == "act":
                nc.scalar.activation(ot[:], po[:], AF.Copy, scale=PROJ_SCALE)
            elif eng == "pool":
                nc.gpsimd.tensor_scalar(ot[:], po[:], PROJ_SCALE, None,
                                        ALU.mult)
            else:
                nc.vector.tensor_scalar(ot[:], po[:], PROJ_SCALE, None,
                                        ALU.mult)
            getattr(nc, dmaq).dma_start(outT[mt * 128:(mt + 1) * 128, sl],
                                        ot[:])

        def emit_proj_pair(mt, pair, engs=("dve", "dve"), dmaq="gpsimd"):
            ot = outp.tile([128, 2, TCH], BF16, tag="ot")
            for half in range(2):
                tc8 = pair * 2 + half
                sl = slice(tc8 * TCH, (tc8 + 1) * TCH)
                po = psA.tile([128, TCH], F32, tag="a", name=f"po{mt}_{tc8}")
                for cht in range(2):
                    nc.tensor.matmul(po[:],
                                     wp_sb[:, cht, mt * 128:(mt + 1) * 128],
                                     yn[:, cht, sl], start=(cht == 0),
                                     stop=(cht == 1))
                if engs[half] == "act":
                    nc.scalar.activation(ot[:, half], po[:], AF.Copy,
                                         scale=PROJ_SCALE)
                elif engs[half] == "pool":
                    nc.gpsimd.tensor_scalar(ot[:, half], po[:], PROJ_SCALE,
                                            None, ALU.mult)
                else:
                    nc.vector.tensor_scalar(ot[:, half], po[:], PROJ_SCALE,
                                            None, ALU.mult)
            getattr(nc, dmaq).dma_start(
                outT[mt * 128:(mt + 1) * 128,
                     pair * 2 * TCH:(pair * 2 + 2) * TCH], ot[:])

        fillers: list = []

        def drain(n):
            # qkv fillers gate future exps: keep them at normal priority.
            # proj fillers are pure sinks: push them to low priority.
            save = tc.cur_priority
            try:
                for _ in range(min(n, len(fillers))):
                    kind, fn = fillers.pop(0)
                    tc.cur_priority = save + {"gate": 8000, "v": 12000,
                                              "sink": 16000}[kind]
                    fn()
            finally:
                tc.cur_priority = save
            return

        ys_tiles: dict = {}

        def emit_attn_head(j2, h, fill_every=2, fill_at=None,
                           split_exp=False):
            kmt = h // 2
            qmt = h // 2
            nkt = 8 * j2 + 8
            q0 = j2 * QW
            yp = psY.tile([128, 8, 128], F32, tag="yp", name=f"yp{j2}_{h}")
            rc = rcp.tile([128, 8], F32, tag="rc", name=f"rc{j2}_{h}")
            p0 = (h % 2) * 64
            if h % 2 == 0:
                ysc = ysp.tile([128, 8, 128], BF16, tag="ys",
                               name=f"ys{j2}_{h // 2}")
                ys_tiles[(j2, h // 2)] = ysc
            else:
                ysc = ys_tiles[(j2, h // 2)]
            ys = ysc[:, :, p0:p0 + HD]
            for kt in range(nkt):
                lead = (kt // 8 == j2)
                cs = 128 * (kt % 8) if lead else 0
                kp = (h % 2) * 64
                k_lhs = khl[kp:kp + 64, kmt, :, kt * 128:(kt + 1) * 128]
                st = psS.tile([128, QW], F32, tag="st", name=f"st{j2}_{h}_{kt}")
                ranges = ((cs, 512), (512, QW)) if cs < 512 else ((cs, QW),)
                for lo, hi in ranges:
                    # each range opens its own PSUM bank group (start=True);
                    # a start=False matmul here would accumulate onto stale
                    # bank contents from the previous st tile use.
                    q_rhs = q8[kp:kp + 64, qmt, q0 + lo:q0 + hi] \
                        .unsqueeze(1).broadcast_to([64, 2, hi - lo])
                    nc.tensor.matmul(st[:, lo:hi], k_lhs, q_rhs,
                                     start=True, stop=True, perf_mode=DR)
                if lead:
                    # causal mask: st[:, diag] += A^T B (-16384 above diag)
                    nc.tensor.matmul(st[:, cs:cs + 128], mask_sb[:, 0, :],
                                     mask_sb[:, 1, :], start=False, stop=True,
                                     skip_group_check=True)
                pt = ptp.tile([128, QW], BF16, tag="pt")
                if split_exp and cs < 512:
                    # halve the first unit's exps so the stream starts as
                    # soon as the first q8 chunk lands
                    nc.scalar.activation(pt[:, cs:512], st[:, cs:512],
                                         AF.Exp, scale=EXP_SCALE)
                    nc.scalar.activation(pt[:, 512:], st[:, 512:], AF.Exp,
                                         scale=EXP_SCALE)
                else:
                    nc.scalar.activation(pt[:, cs:], st[:, cs:], AF.Exp,
                                         scale=EXP_SCALE)
                # PSUM zero regions are bank-wide (2KB): only one accumulation
                # group per bank. Open each bank once (j=0/j=4 at kt=0); the
                # bank-wide pending-zero gives the other subtiles their
                # initial zeroing; close with the bank's last accumulation.
                j0 = max(0, kt - 8 * j2)
                for j in range(j0, 8):
                    nc.tensor.matmul(yp[:, j, 0:HD + 1],
                                     pt[:, j * 128:(j + 1) * 128],
                                     v1[:, h, kt, :],
                                     start=(kt == 0 and j % 4 == 0),
                                     stop=(j % 4 == 3 and kt == 8 * j2 + j))
                if kt == 8 * j2 + 3:
                    # bank 0 (subtiles 0-3) just closed: normalize its half
                    # now, 4 k-tiles before the unit ends
                    nc.vector.reciprocal(rc[:, 0:4], yp[:, 0:4, HD])
                    for j in range(4):
                        nc.vector.tensor_scalar(ys[:, j, :], yp[:, j, 0:HD],
                                                rc[:, j:j + 1], None, ALU.mult)
                if fill_at is not None:
                    if kt in fill_at:
                        drain(1)
                elif (kt + 1) % fill_every == 0:
                    drain(1)
            # bank 1 half (the last head's copies split across DVE/ACT to
            # shorten the post-stream tail)
            nc.vector.reciprocal(rc[:, 4:8], yp[:, 4:8, HD])
            tail_head = (j2 == 1 and h == HPC - 1)
            for j in range(4, 8):
                if tail_head and j % 2 == 1:
                    nc.scalar.activation(ys[:, j, :], yp[:, j, 0:HD],
                                         AF.Copy, scale=rc[:, j:j + 1])
                else:
                    nc.vector.tensor_scalar(ys[:, j, :], yp[:, j, 0:HD],
                                            rc[:, j:j + 1], None, ALU.mult)
            if _DEBUG and h == 0:
                nc.sync.dma_start(t_in["ys_dbg"][:, j2], ys[:])
                nc.sync.dma_start(t_in["rc_dbg"][:, j2], rc[:])

        def emit_dphase_half(j2, cht, half, ysc, eng="dve"):
            # transpose back: yn[ch, tok] = ys[q, ch].T
            # mid-stream phases use the XBAR DMA transpose on the idle SP
            # queue (frees PE + DVE); eng="pe" keeps the identity-matmul path
            # for the latency-critical final phase.
            if eng == "dma":
                for jj in range(4):
                    j = half * 4 + jj
                    t0 = j2 * QW + j * 128
                    nc.sync.dma_start(yn[:, cht, t0:t0 + 128],
                                      ysc[:, j, :], transpose=True)
                return
            ys_pair = [ysc[:, :, 0:HD], ysc[:, :, HD:2 * HD]]
            if True:
                dout = psA.tile([128, 512], F32, tag="a",
                                name=f"do{j2}_{cht}_{half}")
                for hh in range(2):
                    for jj in range(4):
                        j = half * 4 + jj
                        nc.tensor.matmul(dout[hh * 64:(hh + 1) * 64,
                                              jj * 128:(jj + 1) * 128],
                                         ys_pair[hh][:, j, :],
                                         mask_sb[:, 1, :],
                                         start=True, stop=True)
                t0 = j2 * QW + half * 512
                if eng == "act":
                    nc.scalar.activation(yn[:, cht, t0:t0 + 512], dout[:],
                                         AF.Copy, scale=1.0 / MASK_VAL)
                elif eng == "split":
                    nc.vector.tensor_scalar(yn[:, cht, t0:t0 + 256],
                                            dout[:, 0:256], 1.0 / MASK_VAL,
                                            None, ALU.mult)
                    nc.scalar.activation(yn[:, cht, t0 + 256:t0 + 512],
                                         dout[:, 256:512], AF.Copy,
                                         scale=1.0 / MASK_VAL)
                else:
                    nc.vector.tensor_scalar(yn[:, cht, t0:t0 + 512],
                                            dout[:], 1.0 / MASK_VAL, None,
                                            ALU.mult)

        def emit_dphase(j2, cht, engs=("dve", "dve")):
            ysc = ys_tiles.pop((j2, cht))
            for half in range(2):
                emit_dphase_half(j2, cht, half, ysc, engs[half])

        # ---------- schedule ----------
        _mark(nc, "qkv0")
        for tc8 in range(2):
            for mt in (0, 2):   # heads 0/1 q+k; ACT is idle before attention
                emit_qk_chunk(tc8, mt, eng="act")
        save_p = tc.cur_priority
        tc.cur_priority = save_p + 12000
        for kt in range(4):
            emit_v_chunk(kt)
        tc.cur_priority = save_p

        if "attn" not in _ABLATE:
            # Interleave ACT-light (j2=0) and ACT-heavy (j2=1) units so the
            # exp stream never starves regionally; fillers sized per unit.
            def qkf(tc8, mt):
                fillers.append(("gate", lambda: emit_qk_chunk(tc8, mt)))

            def vf(kt):
                fillers.append(("v", lambda: emit_v_chunk(kt)))

            _mark(nc, "attn0")
            qkf(2, 0); qkf(3, 0)
            for kt in range(4, 8):
                vf(kt)
            qkf(0, 1); qkf(0, 3)
            with tc.high_priority(offset=4000):
                emit_attn_head(0, 0, fill_at=set(range(8)), split_exp=True)
            qkf(2, 2); qkf(3, 2); qkf(1, 1); qkf(1, 3)
            with tc.high_priority(offset=4000):
                emit_attn_head(0, 1, fill_at={0, 1, 2, 3}, split_exp=True)
            emit_dphase(0, 0, engs=("dma", "dma"))
            for kt in range(8, 16):
                vf(kt)
            with tc.high_priority(offset=4000):
                emit_attn_head(1, 0, fill_at=set(range(8)))
            qkf(2, 1); qkf(3, 1); qkf(2, 3); qkf(3, 3)
            with tc.high_priority(offset=4000):
                emit_attn_head(0, 2, fill_at={1, 3, 5, 7})
            with tc.high_priority(offset=4000):
                emit_attn_head(1, 1, fill_at={3, 7, 11, 15})
            emit_dphase(1, 0, engs=("dma", "dma"))
            with tc.high_priority(offset=4000):
                emit_attn_head(0, 3, fill_at={1, 3, 5, 7})
            _mark(nc, "dphase0")
            drain(len(fillers))
            emit_dphase(0, 1, engs=("dma", "dma"))
            _mark(nc, "attn1")
            for mt in range(NCT - 2):
                fillers.append(("sink", lambda mt=mt: emit_proj_pair(mt, 0)))
            with tc.high_priority(offset=4000):
                emit_attn_head(1, 2, fill_at={1, 5, 9, 13})
            with tc.high_priority(offset=4000):
                emit_attn_head(1, 3, fill_at={1, 5, 9, 13})
            _mark(nc, "dphase1")
            for mt in (NCT - 2, NCT - 1):
                emit_proj_pair(mt, 0)
            drain(len(fillers))
            ysc_t = ys_tiles.pop((1, 1))
            with tc.high_priority(offset=4000):
                emit_dphase_half(1, 1, 0, ysc_t, "dve")
            for i, mt in enumerate(range(NCT)):
                emit_proj_single(mt, 2, eng=("act", "dve")[i % 2],
                                 dmaq=("sync", "gpsimd")[i % 2],
                                 pool=(None, psS)[i % 2])
            with tc.high_priority(offset=4000):
                emit_dphase_half(1, 1, 1, ysc_t, "split")
            for i, mt in enumerate(range(NCT)):
                emit_proj_single(mt, 3, eng=("dve", "act")[i % 2],
                                 dmaq=("gpsimd", "sync")[i % 2],
                                 pool=(None, psS)[i % 2])
        else:
            for tc8 in range(2):
                for mt in (1, 3):
                    emit_qk_chunk(tc8, mt)
            for tc8 in range(2, 4):
                for mt in range(NMT):
                    emit_qk_chunk(tc8, mt)
            for kt in range(8, 16):
                emit_v_chunk(kt)
            for mt in range(NCT):
                emit_proj_pair(mt, 0)

        _mark(nc, "projtail")
        if "proj" not in _ABLATE and "attn" in _ABLATE:
            engs = [("dve", "act"), ("act", "dve")]
            for mt in range(NCT):
                emit_proj_pair(mt, 1, engs=engs[mt % 2], dmaq="sync")

        if _DEBUG:
            nc.sync.dma_start(t_in["q8_dbg"][:], q8[:])
            nc.sync.dma_start(t_in["khl_dbg"][:], khl[:])
            nc.sync.dma_start(t_in["v1_dbg"][:], v1[:])
            nc.sync.dma_start(t_in["yn_dbg"][:], yn[:])


def _declare_io(nc):
    t_in = {
        # [128, tc8, hl, ct, tok] fp8 (hi/lo planes contiguous per chunk)
        "xq": nc.dram_tensor("xq", [128, NTC, 2, NCT, TCH], FP8,
                             kind="ExternalInput"),
        "w_q": nc.dram_tensor("w_q", [128, NCT, 2, 256], FP8,
                              kind="ExternalInput"),
        "w_k": nc.dram_tensor("w_k", [128, NCT, 2, 256], FP8,
                              kind="ExternalInput"),
        "w_v": nc.dram_tensor("w_v", [128, NCT, 2, 256], FP8,
                              kind="ExternalInput"),
        "wp_eff": nc.dram_tensor("wp_eff", [128, 2, C], BF16,
                                 kind="ExternalInput"),
        "masks": nc.dram_tensor("masks", [128, 2, 128], BF16,
                                kind="ExternalInput"),
    }
    outT = nc.dram_tensor("outT", [C, T], BF16, kind="ExternalOutput")
    if _DEBUG:
        t_in["q8_dbg"] = nc.dram_tensor("q8_dbg", [128, 2, T], FP8,
                                        kind="ExternalOutput")
        t_in["khl_dbg"] = nc.dram_tensor("khl_dbg", [128, 2, 2, T], FP8,
                                         kind="ExternalOutput")
        t_in["v1_dbg"] = nc.dram_tensor("v1_dbg", [128, HPC, KT, HD + 1],
                                        BF16, kind="ExternalOutput")
        t_in["yn_dbg"] = nc.dram_tensor("yn_dbg", [128, 2, T], BF16,
                                        kind="ExternalOutput")
        t_in["ys_dbg"] = nc.dram_tensor("ys_dbg", [128, 2, 8, HD], BF16,
                                        kind="ExternalOutput")
        t_in["rc_dbg"] = nc.dram_tensor("rc_dbg", [128, 2, 8], F32,
                                        kind="ExternalOutput")
    return t_in, outT


def _build(reps: int = 1):
    nc = bacc.Bacc("TRN2", target_bir_lowering=False, debug=False)
    t_in, outT = _declare_io(nc)
    with tile.TileContext(nc) as tc:
        with ExitStack() as ctx:
            _emit(ctx, tc, t_in, outT, reps=reps)
    nc.compile()
    return nc


def _fp8_split(a: np.ndarray):
    """Return (hi, lo) fp8e4m3 pair with hi + lo ~= a."""
    f8 = ml_dtypes.float8_e4m3
    hi = a.astype(f8)
    lo = (a - hi.astype(np.float32)).astype(f8)
    return hi, lo


def _make_in_maps(inputs: dict) -> list:
    f32 = np.float32
    f8 = ml_dtypes.float8_e4m3
    x = np.asarray(inputs["x"], f32)                     # [B, T, C]
    w_attn = np.asarray(inputs["w_attn"], f32)
    la_attn = np.ascontiguousarray(np.asarray(inputs["la_attn"], f32))
    lb_attn = np.asarray(inputs["lb_attn"], f32)
    w_proj = np.asarray(inputs["w_proj"], f32)
    la_proj = np.asarray(inputs["la_proj"], f32)
    lb_proj = np.asarray(inputs["lb_proj"], f32)

    # fold LoRA into effective weights on the host (input preprocessing)
    Wq = w_attn + 0.5 * lb_attn @ la_attn                # [3C, C]
    Wp = w_proj + 0.5 * lb_proj @ la_proj                # [C, C]

    # x: [128, tc8, hl, ct, tok] fp8, scaled by XS
    xq_b = []
    for b in range(B):
        xT = np.ascontiguousarray(x[b].T) * XS           # [C, T]
        xr = xT.reshape(NCT, 128, T)                     # [ct, p, t]
        hi, lo = _fp8_split(xr)
        arr = np.empty((128, NTC, 2, NCT, TCH), f8)
        for tc8 in range(NTC):
            sl = slice(tc8 * TCH, (tc8 + 1) * TCH)
            arr[:, tc8, 0] = hi[:, :, sl].transpose(1, 0, 2)
            arr[:, tc8, 1] = lo[:, :, sl].transpose(1, 0, 2)
        xq_b.append(arr)

    k_idx = np.arange(128)[:, None]
    q_idx = np.arange(128)[None, :]
    masks = np.zeros((128, 2, 128), ml_dtypes.bfloat16)
    masks[:, 0, :] = (q_idx > k_idx)         # A[j, k] = 1 iff k > j
    masks[:, 1, :] = MASK_VAL * (q_idx == k_idx)

    in_maps = []
    for core in range(NCORES):
        b, g = core // 4, core % 4
        ch0 = g * CH
        rows = np.r_[ch0:ch0 + CH, C + ch0:C + ch0 + CH,
                     2 * C + ch0:2 * C + ch0 + CH]
        # [p, ct, hl, r] = fp8 split of (WS * Wq).T[ct*128+p, r], per row block
        wq_s = np.ascontiguousarray(
            (WS * Wq[rows]).T.reshape(NCT, 128, NQR).transpose(1, 0, 2))
        w_hi, w_lo = _fp8_split(wq_s)                    # [p, ct, 768]
        whl = np.stack([w_hi, w_lo], axis=2)             # [p, ct, 2, 768]
        # [p, cht, c] = Wp.T[ch0+cht*128+p, c]
        wp_eff = np.ascontiguousarray(
            Wp[:, ch0:ch0 + CH].T.reshape(2, 128, C).transpose(1, 0, 2)
        ).astype(ml_dtypes.bfloat16)
        in_maps.append({
            "xq": xq_b[b],
            "w_q": np.ascontiguousarray(whl[:, :, :, 0:256]),
            "w_k": np.ascontiguousarray(whl[:, :, :, 256:512]),
            "w_v": np.ascontiguousarray(whl[:, :, :, 512:768]),
            "wp_eff": wp_eff,
            "masks": masks,
        })
    return in_maps


def _execute(inputs: dict, trace: bool = False):
    if "nc" not in _CACHE:
        _CACHE["nc"] = _build()
    nc = _CACHE["nc"]
    in_maps = _make_in_maps(inputs)
    res = run_bass_kernel_spmd(nc, in_maps, core_ids=list(range(NCORES)),
                               trace=trace)
    out = np.empty((B, T, C), np.float32)
    for b in range(B):
        acc = np.zeros((C, T), np.float32)
        for g in range(4):
            acc += np.asarray(res.results[b * 4 + g]["outT"], dtype=np.float32)
        out[b] = acc.T
    return out, res


def kernel(**inputs) -> np.ndarray:
    out, _ = _execute(inputs, trace=False)
    return out


# revision 48
# speedup vs baseline: 1.0861x; 1.0557x over previous
"""Trainium2 Bass kernel for a causal self-attention block with LoRA adapters.

Model (B=2, T=2048, C=1024, H=16 heads, hd=64, LoRA r=32, scale 0.5):
    qkv = x @ w_attn.T + 0.5*(x @ la_attn.T) @ lb_attn.T      (biases are 0)
    y   = causal_softmax_attention(q, k, v)
    out = y @ w_proj.T + 0.5*(y @ la_proj.T) @ lb_proj.T

Sharding: 8 cores = 2 batches x 4 head-groups. Core c owns batch c//4 and
heads 4*(c%4)..4*(c%4)+3: column-split c_attn (its 768 q/k/v rows over its
batch's 2048 tokens), full attention for its 4 heads, row-split c_proj
producing a 4-way partial [C, T]; the host sums 4 partials per batch.

Device algorithm per core (fp32 PSUM everywhere):
  - LoRA folded into effective weights on the host.  The big GEMMs use
    fp8e4m3 DoubleRow matmuls (0.5 PE rows/cycle, 256-deep contraction):
    * qkv: x and W shipped as (hi, lo) fp8 pairs (x_s = 4x, W_s = 128W);
      3-term product Whi*(xhi+xlo) + Wlo*xhi.  The hi/lo pair rides dim1 of
      one DoubleRow matmul with the other operand broadcast (0-stride), so
      the 3 terms cost 6 bf16-equivalent passes instead of 8.
    * S = q.k: k is stored as an (hi, lo) fp8 pair; one DoubleRow matmul
      per key-tile contracts [k_hi; k_lo] x [q; q] (broadcast) -> k at full
      precision, only q carries fp8 rounding.  Half the bf16 PE time.
    * causal mask: folded into the S accumulation group as one extra bf16
      matmul  st[:,diag] += A^T B  (A = strict upper ones, B = -16384*I),
      so exp() gives exact zeros and no per-block mask multiply is needed.
    * AV and c_proj stay bf16 (P cannot be quantized to fp8 cheaply).
  - attention per (j2: 1024-wide q chunk, h): S^T[k, q] blocks into PSUM,
    P = exp(S * 2^-7) on ScalarE; AV in [q, d] orientation with a 0.25
    column appended to v so yp[:,64] = den/4 and ys = yp * (4/den) = 4y.
  - normalize while tokens are on partitions (DVE reciprocal + 8 scaled
    PSUM->SBUF copies), transpose back to [ch, tok] via XBAR DMA transpose
    (mid-stream) or PE identity matmul (latency-critical tail).
  - outT_partial = Wp^T @ yn per 128-channel tile, *0.25 fused into the
    PSUM->SBUF copies.  Schedule: qkv/proj chunks drain into PE gaps in
    priority bands so neither PE nor the ScalarE exp stream starves.
Output: bf16 partial [C, T] per core; host sums 4 partials per batch in f32.
"""

from contextlib import ExitStack

import numpy as np
import ml_dtypes

import concourse.bass as bass
import concourse.tile as tile
from concourse import bacc, mybir
from concourse.bass_utils import run_bass_kernel_spmd

F32 = mybir.dt.float32
BF16 = mybir.dt.bfloat16
FP8 = mybir.dt.float8e4
AF = mybir.ActivationFunctionType
ALU = mybir.AluOpType
DR = mybir.MatmulPerfMode.DoubleRow

B, T, C, H, R = 2, 2048, 1024, 16, 32
HD = C // H              # 64
NCORES = 8
HPC = 4                  # heads per core
CH = HPC * HD            # 256 per-core channels
NCT = C // 128           # 8 contraction tiles
NQR = 3 * CH             # 768 qkv rows per core
NMT = 2 * CH // 128      # 4 q+k partition tiles
KT = T // 128            # 16 key tiles
QW = 1024                # q chunk width
TCH = 512                # token chunk for qkv/proj
NTC = T // TCH           # 4

XS = 4.0                 # host scale on x
WS = 128.0               # host scale on w_attn
QKV_SCALE = 2.0 ** -7    # psum (= 512 * raw) -> 4 * raw for q/k fp8
V_SCALE = 2.0 ** -9      # psum -> raw for v (bf16)
EXP_SCALE = 2.0 ** -7    # S_psum = 16 * S_raw; want exp(S_raw / 8)
ONES_VAL = 0.25          # v denominator column -> ys = 4 * y
PROJ_SCALE = 0.25        # proj psum (= 4 * out) -> out
MASK_VAL = -16384.0      # masked S entries (exp -> 0)

_CACHE: dict = {}
_PHASE_MARKS: list = []
_ABLATE: set = set()
_DEBUG = False


def _mark(nc, name):
    _PHASE_MARKS.append((name, nc.next_id()))


def _emit(ctx: ExitStack, tc: tile.TileContext, t_in: dict, outT, reps: int = 1):
    nc = tc.nc
    _PHASE_MARKS.clear()
    _mark(nc, "setup")

    singles = ctx.enter_context(tc.tile_pool(name="singles", bufs=1))
    psS = ctx.enter_context(tc.tile_pool(name="psS", bufs=2, space=bass.MemorySpace.PSUM))
    psY = ctx.enter_context(tc.tile_pool(name="psY", bufs=1, space=bass.MemorySpace.PSUM))
    psA = ctx.enter_context(tc.tile_pool(name="psA", bufs=2, space=bass.MemorySpace.PSUM))
    ptp = ctx.enter_context(tc.tile_pool(name="ptp", bufs=24))
    ysp = ctx.enter_context(tc.tile_pool(name="ysp", bufs=8))
    rcp = ctx.enter_context(tc.tile_pool(name="rcp", bufs=8))
    outp = ctx.enter_context(tc.tile_pool(name="outp", bufs=8))

    # ---------- constants / weights to SBUF ----------
    # x ships as fp8 (hi, lo) pairs laid out [128, ct, hl, tok] per 512-token
    # chunk; weights as fp8 hi + lo planes. Queues: scalar (ACT) carries
    # weights, sync (SP) the x head, gpsimd (Pool SWDGE) the x tail.
    # x per 512-token chunk, hi and lo planes in SEPARATE tiles: DMA-write
    # -> compute-read dependencies are tile-granular, so the hi-only T1+T3
    # matmuls must not share a tile with the later-arriving lo plane.
    xh = [singles.tile([128, NCT, TCH], FP8, name=f"xh{i}")
          for i in range(NTC)]
    xl = [singles.tile([128, NCT, TCH], FP8, name=f"xl{i}")
          for i in range(NTC)]
    # weights per row-block, (hi, lo) interleaved on dim2: one DoubleRow
    # matmul per c computes Whi*xhi + Wlo*xhi with the pair as stationary;
    # the Whi*xlo correction pairs c-planes (stride-2 APs into the same
    # tiles).
    wq_sb = [singles.tile([128, NCT, 2, 128], FP8, name=f"wq{m}")
             for m in range(2)]                    # q rows per head-pair
    wk_sb = [singles.tile([128, NCT, 2, 128], FP8, name=f"wk{m}")
             for m in range(2)]                    # k rows per head-pair
    wv_sb = singles.tile([128, NCT, 2, 256], FP8)  # v rows 512:768
    wp_sb = singles.tile([128, 2, C], BF16)
    mask_sb = singles.tile([128, 2, 128], BF16)  # [:,0]=A ones, [:,1]=B diag

    _mark(nc, "xload")

    def xload(tc8, hl, queue):
        queue.dma_start((xh if hl == 0 else xl)[tc8][:],
                        t_in["xq"][:, tc8, hl])

    # Critical-order loading: the first attention unit needs (in order)
    # wk/wq rows for heads 0/1, x chunk0 hi, then the lo planes.  sync and
    # scalar HWDGE queues interleave on the single DMA-engine pool, so the
    # emission order here IS the landing order.  The x tail (chunks 2-3)
    # goes on the Pool SWDGE queue but is emitted later (at attn0) so its
    # transfers don't steal DMA-engine slots from the critical pieces.
    nc.scalar.dma_start(mask_sb[:], t_in["masks"][:])
    nc.scalar.dma_start(wk_sb[0][:], t_in["w_k0"][:])
    if "xload" not in _ABLATE:
        xload(0, 0, nc.sync)
        xload(0, 1, nc.sync)
    nc.scalar.dma_start(wq_sb[0][:], t_in["w_q0"][:])
    nc.scalar.dma_start(wk_sb[1][:], t_in["w_k1"][:])
    if "xload" not in _ABLATE:
        xload(1, 0, nc.sync)
    nc.scalar.dma_start(wq_sb[1][:], t_in["w_q1"][:])
    if "xload" not in _ABLATE:
        xload(1, 1, nc.sync)
    nc.scalar.dma_start(wv_sb[:], t_in["w_v"][:])
    nc.scalar.dma_start(wp_sb[:], t_in["wp_eff"][:])

    def xload_tail():
        if "xload" not in _ABLATE:
            # The Pool queue is otherwise empty, so its SWDGE loads would
            # fire at t=0 and steal DMA-engine slots from the critical
            # head pieces.  Gate each tail DMA behind the last critical x
            # piece by first writing its dest tile with a tiny Pool copy
            # that reads xl[1] (write->write ordering is tile-granular).
            for tc8 in range(2, 4):
                for hl in range(2):
                    dst = (xh if hl == 0 else xl)[tc8]
                    nc.gpsimd.tensor_copy(dst[0:1, 0, 0:8],
                                          xl[1][0:1, 0, 0:8])
                    xload(tc8, hl, nc.gpsimd)

    # PE p-state warmup: the clock ramps only while the engine is
    # continuously busy (>3us to reach 2.4GHz), so spin defined-value
    # matmuls into a scratch PSUM bank while the first x/w DMAs are in
    # flight.  Sized to end just as the first real chunk's inputs land.
    warm_sb = singles.tile([128, TCH], BF16)
    nc.vector.memset(warm_sb[:], 0.0)

    for _rep in range(reps):
        q8 = singles.tile([128, 2, T], FP8)        # q as fp8, 4*q_raw
        khl = singles.tile([128, 2, 2, T], FP8)    # k (hi, lo) pairs
        v1 = singles.tile([128, HPC, KT, HD + 1], BF16)
        nc.vector.memset(v1[:, :, :, HD:HD + 1], ONES_VAL)
        warm_ps = psY.tile([128, 8, 128], F32, tag="yp", name="warm")
        for wi in range(10):
            nc.tensor.matmul(warm_ps[:, 0:4, :], warm_sb[:, 0:128],
                             warm_sb[:], start=True, stop=True)
        # yn.T = 4*y per channel tile, split per (j2, cht) so a proj chunk
        # only depends on the dphase DMA writes of its own token half
        # (DMA-write -> read deps are tile-granular).
        yn = {(j2, cht): singles.tile([128, QW], BF16, name=f"yn{j2}{cht}")
              for j2 in range(2) for cht in range(2)}
        if "attn" in _ABLATE:
            for t in yn.values():
                nc.vector.memset(t[:], 1.0)

        def qkv_matmuls(ps, tc8, wt, rows, tok=slice(0, TCH),
                        x_station=False):
            """12 DoubleRow matmuls: (Whi+Wlo)*xhi + Whi*xlo.

            x-stationary (v) flavor uses c-pair form throughout so the hi
            and lo planes stay in separate tiles:
              (xhi[c],xhi[c+1])x(Whi[c],Whi[c+1]) + same x(Wlo..) + xlo x Whi
            """
            if x_station:
                for t13 in range(2):   # 0: xhi*Whi pairs, 1: xhi*Wlo pairs
                    for cp in range(NCT // 2):
                        cs = slice(2 * cp, 2 * cp + 2)
                        nc.tensor.matmul(
                            ps[:], xh[tc8][:, cs, tok], wt[:, cs, t13, rows],
                            start=(t13 == 0 and cp == 0), stop=False,
                            perf_mode=DR)
                for cp in range(NCT // 2):
                    cs = slice(2 * cp, 2 * cp + 2)
                    nc.tensor.matmul(
                        ps[:], xl[tc8][:, cs, tok], wt[:, cs, 0, rows],
                        start=False, stop=(cp == NCT // 2 - 1), perf_mode=DR)
                return
            for c in range(NCT):
                # T1+T3: stationary (Whi[c], Wlo[c]) pair x broadcast xhi[c]
                nc.tensor.matmul(
                    ps[:], wt[:, c, :, rows],
                    xh[tc8][:, c, tok].unsqueeze(1).broadcast_to(
                        [128, 2, TCH]),
                    start=(c == 0), stop=False, perf_mode=DR)
            for cp in range(NCT // 2):
                # T2: (Whi[c], Whi[c+1]) pair x (xlo[c], xlo[c+1]) pair
                cs = slice(2 * cp, 2 * cp + 2)
                nc.tensor.matmul(ps[:], wt[:, cs, 0, rows],
                                 xl[tc8][:, cs, tok], start=False,
                                 stop=(cp == NCT // 2 - 1), perf_mode=DR)

        def emit_qk_chunk(tc8, mt, eng="dve"):
            sl = slice(tc8 * TCH, (tc8 + 1) * TCH)
            ps = psA.tile([128, TCH], F32, tag="a", name=f"qk{tc8}_{mt}")
            wt = (wq_sb[mt] if mt < 2 else wk_sb[mt - 2])
            qkv_matmuls(ps, tc8, wt, slice(0, 128))
            if mt < 2:  # q -> single fp8
                if eng == "act":
                    nc.scalar.activation(q8[:, mt, sl], ps[:], AF.Copy,
                                         scale=QKV_SCALE)
                else:
                    nc.vector.tensor_scalar(q8[:, mt, sl], ps[:],
                                            QKV_SCALE, None, ALU.mult)
            else:       # k -> (hi, lo) fp8 pair
                kh = khl[:, mt - 2, 0, sl]
                if eng == "act":
                    nc.scalar.activation(kh, ps[:], AF.Copy, scale=QKV_SCALE)
                else:
                    nc.vector.tensor_scalar(kh, ps[:], QKV_SCALE, None,
                                            ALU.mult)
                nc.vector.scalar_tensor_tensor(
                    khl[:, mt - 2, 1, sl], ps[:], QKV_SCALE, kh,
                    ALU.mult, ALU.subtract)

        def emit_v_chunk(kt):
            ps = psA.tile([128, CH], F32, tag="a", name=f"v{kt}",
                          padded_shape=[128, 512])
            qkv_matmuls(ps, kt // 4, wv_sb, slice(0, CH),
                        tok=slice((kt % 4) * 128, (kt % 4 + 1) * 128),
                        x_station=True)
            nc.vector.tensor_scalar(
                v1[:, :, kt, 0:HD],
                ps[:].rearrange("p (h d) -> p h d", h=HPC),
                V_SCALE, None, ALU.mult)

        def emit_proj_single(mt, tc8, eng="dve", dmaq="sync", pool=None):
            sl = slice(tc8 * TCH, (tc8 + 1) * TCH)
            po = (pool or psA).tile([128, TCH], F32,
                                    tag="a" if pool is None else "st",
                                    name=f"po{mt}_{tc8}")
            lsl = slice((tc8 % 2) * TCH, (tc8 % 2 + 1) * TCH)
            for cht in range(2):
                nc.tensor.matmul(po[:],
                                 wp_sb[:, cht, mt * 128:(mt + 1) * 128],
                                 yn[(tc8 // 2, cht)][:, lsl],
                                 start=(cht == 0), stop=(cht == 1))
            ot = outp.tile([128, TCH], BF16, tag="ots")
            if eng == "act":
                nc.scalar.activation(ot[:], po[:], AF.Copy, scale=PROJ_SCALE)
            elif eng == "pool":
                nc.gpsimd.tensor_scalar(ot[:], po[:], PROJ_SCALE, None,
                                        ALU.mult)
            else:
                nc.vector.tensor_scalar(ot[:], po[:], PROJ_SCALE, None,
                                        ALU.mult)
            getattr(nc, dmaq).dma_start(outT[mt * 128:(mt + 1) * 128, sl],
                                        ot[:])

        def emit_proj_pair(mt, pair, engs=("dve", "dve"), dmaq="gpsimd"):
            ot = outp.tile([128, 2, TCH], BF16, tag="ot")
            for half in range(2):
                tc8 = pair * 2 + half
                sl = slice(tc8 * TCH, (tc8 + 1) * TCH)
                po = psA.tile([128, TCH], F32, tag="a", name=f"po{mt}_{tc8}")
                lsl = slice((tc8 % 2) * TCH, (tc8 % 2 + 1) * TCH)
                for cht in range(2):
                    nc.tensor.matmul(po[:],
                                     wp_sb[:, cht, mt * 128:(mt + 1) * 128],
                                     yn[(tc8 // 2, cht)][:, lsl],
                                     start=(cht == 0), stop=(cht == 1))
                if engs[half] == "act":
                    nc.scalar.activation(ot[:, half], po[:], AF.Copy,
                                         scale=PROJ_SCALE)
                elif engs[half] == "pool":
                    nc.gpsimd.tensor_scalar(ot[:, half], po[:], PROJ_SCALE,
                                            None, ALU.mult)
                else:
                    nc.vector.tensor_scalar(ot[:, half], po[:], PROJ_SCALE,
                                            None, ALU.mult)
            getattr(nc, dmaq).dma_start(
                outT[mt * 128:(mt + 1) * 128,
                     pair * 2 * TCH:(pair * 2 + 2) * TCH], ot[:])

        fillers: list = []

        def drain(n):
            # qkv fillers gate future exps: keep them at normal priority.
            # proj fillers are pure sinks: push them to low priority.
            save = tc.cur_priority
            try:
                for _ in range(min(n, len(fillers))):
                    kind, fn = fillers.pop(0)
                    tc.cur_priority = save + {"gate": 8000, "v": 12000,
                                              "sink": 16000}[kind]
                    fn()
            finally:
                tc.cur_priority = save
            return

        ys_tiles: dict = {}

        def emit_attn_head(j2, h, fill_every=2, fill_at=None,
                           split_exp=False):
            kmt = h // 2
            qmt = h // 2
            nkt = 8 * j2 + 8
            q0 = j2 * QW
            yp = psY.tile([128, 8, 128], F32, tag="yp", name=f"yp{j2}_{h}")
            rc = rcp.tile([128, 8], F32, tag="rc", name=f"rc{j2}_{h}")
            p0 = (h % 2) * 64
            if h % 2 == 0:
                ysc = ysp.tile([128, 8, 128], BF16, tag="ys",
                               name=f"ys{j2}_{h // 2}")
                ys_tiles[(j2, h // 2)] = ysc
            else:
                ysc = ys_tiles[(j2, h // 2)]
            ys = ysc[:, :, p0:p0 + HD]
            for kt in range(nkt):
                lead = (kt // 8 == j2)
                cs = 128 * (kt % 8) if lead else 0
                kp = (h % 2) * 64
                k_lhs = khl[kp:kp + 64, kmt, :, kt * 128:(kt + 1) * 128]
                st = psS.tile([128, QW], F32, tag="st", name=f"st{j2}_{h}_{kt}")
                ranges = ((cs, 512), (512, QW)) if cs < 512 else ((cs, QW),)
                for lo, hi in ranges:
                    # each range opens its own PSUM bank group (start=True);
                    # a start=False matmul here would accumulate onto stale
                    # bank contents from the previous st tile use.
                    q_rhs = q8[kp:kp + 64, qmt, q0 + lo:q0 + hi] \
                        .unsqueeze(1).broadcast_to([64, 2, hi - lo])
                    nc.tensor.matmul(st[:, lo:hi], k_lhs, q_rhs,
                                     start=True, stop=True, perf_mode=DR)
                if lead:
                    # causal mask: st[:, diag] += A^T B (-16384 above diag)
                    nc.tensor.matmul(st[:, cs:cs + 128], mask_sb[:, 0, :],
                                     mask_sb[:, 1, :], start=False, stop=True,
                                     skip_group_check=True)
                pt = ptp.tile([128, QW], BF16, tag="pt")
                if split_exp and cs < 512:
                    # halve the first unit's exps so the stream starts as
                    # soon as the first q8 chunk lands
                    nc.scalar.activation(pt[:, cs:512], st[:, cs:512],
                                         AF.Exp, scale=EXP_SCALE)
                    nc.scalar.activation(pt[:, 512:], st[:, 512:], AF.Exp,
                                         scale=EXP_SCALE)
                else:
                    nc.scalar.activation(pt[:, cs:], st[:, cs:], AF.Exp,
                                         scale=EXP_SCALE)
                # PSUM zero regions are bank-wide (2KB): only one accumulation
                # group per bank. Open each bank once (j=0/j=4 at kt=0); the
                # bank-wide pending-zero gives the other subtiles their
                # initial zeroing; close with the bank's last accumulation.
                j0 = max(0, kt - 8 * j2)
                for j in range(j0, 8):
                    nc.tensor.matmul(yp[:, j, 0:HD + 1],
                                     pt[:, j * 128:(j + 1) * 128],
                                     v1[:, h, kt, :],
                                     start=(kt == 0 and j % 4 == 0),
                                     stop=(j % 4 == 3 and kt == 8 * j2 + j))
                if kt == 8 * j2 + 3:
                    # bank 0 (subtiles 0-3) just closed: normalize its half
                    # now, 4 k-tiles before the unit ends
                    nc.vector.reciprocal(rc[:, 0:4], yp[:, 0:4, HD])
                    for j in range(4):
                        nc.vector.tensor_scalar(ys[:, j, :], yp[:, j, 0:HD],
                                                rc[:, j:j + 1], None, ALU.mult)
                if fill_at is not None:
                    if kt in fill_at:
                        drain(1)
                elif (kt + 1) % fill_every == 0:
                    drain(1)
            # bank 1 half (the last head's copies split across DVE/ACT to
            # shorten the post-stream tail)
            nc.vector.reciprocal(rc[:, 4:8], yp[:, 4:8, HD])
            tail_head = (j2 == 1 and h == HPC - 1)
            for j in range(4, 8):
                if tail_head and j % 2 == 1:
                    nc.scalar.activation(ys[:, j, :], yp[:, j, 0:HD],
                                         AF.Copy, scale=rc[:, j:j + 1])
                else:
                    nc.vector.tensor_scalar(ys[:, j, :], yp[:, j, 0:HD],
                                            rc[:, j:j + 1], None, ALU.mult)
            if _DEBUG and h == 0:
                nc.sync.dma_start(t_in["ys_dbg"][:, j2], ys[:])
                nc.sync.dma_start(t_in["rc_dbg"][:, j2], rc[:])

        def emit_dphase_half(j2, cht, half, ysc, eng="dve"):
            # transpose back: yn[ch, tok] = ys[q, ch].T
            # mid-stream phases use the XBAR DMA transpose on the idle SP
            # queue (frees PE + DVE); eng="pe" keeps the identity-matmul path
            # for the latency-critical final phase.
            if eng == "dma":
                for jj in range(4):
                    j = half * 4 + jj
                    t0 = j * 128
                    nc.sync.dma_start(yn[(j2, cht)][:, t0:t0 + 128],
                                      ysc[:, j, :], transpose=True)
                return
            ys_pair = [ysc[:, :, 0:HD], ysc[:, :, HD:2 * HD]]
            if True:
                dout = psA.tile([128, 512], F32, tag="a",
                                name=f"do{j2}_{cht}_{half}")
                for hh in range(2):
                    for jj in range(4):
                        j = half * 4 + jj
                        nc.tensor.matmul(dout[hh * 64:(hh + 1) * 64,
                                              jj * 128:(jj + 1) * 128],
                                         ys_pair[hh][:, j, :],
                                         mask_sb[:, 1, :],
                                         start=True, stop=True)
                t0 = half * 512
                ynt = yn[(j2, cht)]
                if eng == "act":
                    nc.scalar.activation(ynt[:, t0:t0 + 512], dout[:],
                                         AF.Copy, scale=1.0 / MASK_VAL)
                elif eng == "split":
                    nc.vector.tensor_scalar(ynt[:, t0:t0 + 256],
                                            dout[:, 0:256], 1.0 / MASK_VAL,
                                            None, ALU.mult)
                    nc.scalar.activation(ynt[:, t0 + 256:t0 + 512],
                                         dout[:, 256:512], AF.Copy,
                                         scale=1.0 / MASK_VAL)
                else:
                    nc.vector.tensor_scalar(ynt[:, t0:t0 + 512],
                                            dout[:], 1.0 / MASK_VAL, None,
                                            ALU.mult)

        def emit_dphase(j2, cht, engs=("dve", "dve")):
            ysc = ys_tiles.pop((j2, cht))
            for half in range(2):
                emit_dphase_half(j2, cht, half, ysc, engs[half])

        # ---------- schedule ----------
        _mark(nc, "qkv0")
        for tc8 in range(2):
            for mt in (2, 0):   # k first: the k hi+lo copies gate the
                emit_qk_chunk(tc8, mt, eng="act")  # first S matmul
        save_p = tc.cur_priority
        tc.cur_priority = save_p + 12000
        for kt in range(4):
            emit_v_chunk(kt)
        tc.cur_priority = save_p

        if "attn" not in _ABLATE:
            # All four j2=0 units run early so dphase(0,*) completes and the
            # 16 proj chunks for tokens 0:1024 become PE filler for the
            # three final j2=1 units; fillers sized per unit.
            def qkf(tc8, mt):
                fillers.append(("gate", lambda: emit_qk_chunk(tc8, mt)))

            def vf(kt):
                fillers.append(("v", lambda: emit_v_chunk(kt)))

            _mark(nc, "attn0")
            xload_tail()
            qkf(2, 0); qkf(3, 0)
            for kt in range(4, 8):
                vf(kt)
            qkf(0, 1); qkf(0, 3)
            with tc.high_priority(offset=4000):
                emit_attn_head(0, 0, fill_at=set(range(8)), split_exp=True)
            qkf(2, 2); qkf(3, 2); qkf(1, 1); qkf(1, 3)
            with tc.high_priority(offset=4000):
                emit_attn_head(0, 1, fill_at={0, 1, 2, 3}, split_exp=True)
            emit_dphase(0, 0, engs=("dma", "dma"))
            for kt in range(8, 16):
                vf(kt)
            with tc.high_priority(offset=4000):
                emit_attn_head(1, 0, fill_at=set(range(8)))
            qkf(2, 1); qkf(3, 1); qkf(2, 3); qkf(3, 3)
            with tc.high_priority(offset=4000):
                emit_attn_head(0, 2, fill_at={1, 3, 5, 7})
            with tc.high_priority(offset=4000):
                emit_attn_head(0, 3, fill_at={1, 3, 5, 7})
            emit_dphase(0, 1, engs=("dma", "dma"))
            _mark(nc, "attn1")
            for mt in range(NCT):
                fillers.append(("sink", lambda mt=mt: emit_proj_pair(mt, 0)))
            with tc.high_priority(offset=4000):
                emit_attn_head(1, 1, fill_at={3, 7, 11, 15})
            emit_dphase(1, 0, engs=("dma", "dma"))
            with tc.high_priority(offset=4000):
                emit_attn_head(1, 2, fill_at={1, 5, 9, 13})
            with tc.high_priority(offset=4000):
                emit_attn_head(1, 3, fill_at={1, 5, 9, 13})
            _mark(nc, "dphase1")
            drain(len(fillers))
            ysc_t = ys_tiles.pop((1, 1))
            with tc.high_priority(offset=4000):
                emit_dphase_half(1, 1, 0, ysc_t, "dve")
            for i, mt in enumerate(range(NCT)):
                emit_proj_single(mt, 2, eng=("act", "dve")[i % 2],
                                 dmaq=("sync", "gpsimd")[i % 2],
                                 pool=(None, psS)[i % 2])
            with tc.high_priority(offset=4000):
                emit_dphase_half(1, 1, 1, ysc_t, "split")
            for i, mt in enumerate(range(NCT)):
                emit_proj_single(mt, 3, eng=("dve", "act")[i % 2],
                                 dmaq=("gpsimd", "sync")[i % 2],
                                 pool=(None, psS)[i % 2])
        else:
            for tc8 in range(2):
                for mt in (1, 3):
                    emit_qk_chunk(tc8, mt)
            for tc8 in range(2, 4):
                for mt in range(NMT):
                    emit_qk_chunk(tc8, mt)
            for kt in range(8, 16):
                emit_v_chunk(kt)
            for mt in range(NCT):
                emit_proj_pair(mt, 0)

        _mark(nc, "projtail")
        if "proj" not in _ABLATE and "attn" in _ABLATE:
            engs = [("dve", "act"), ("act", "dve")]
            for mt in range(NCT):
                emit_proj_pair(mt, 1, engs=engs[mt % 2], dmaq="sync")

        if _DEBUG:
            nc.sync.dma_start(t_in["q8_dbg"][:], q8[:])
            nc.sync.dma_start(t_in["khl_dbg"][:], khl[:])
            nc.sync.dma_start(t_in["v1_dbg"][:], v1[:])
            for j2 in range(2):
                for cht in range(2):
                    nc.sync.dma_start(
                        t_in["yn_dbg"][:, cht, j2 * QW:(j2 + 1) * QW],
                        yn[(j2, cht)][:])


def _declare_io(nc):
    t_in = {
        # [128, tc8, hl, ct, tok] fp8 (hi/lo planes contiguous per chunk)
        "xq": nc.dram_tensor("xq", [128, NTC, 2, NCT, TCH], FP8,
                             kind="ExternalInput"),
        "w_q0": nc.dram_tensor("w_q0", [128, NCT, 2, 128], FP8,
                               kind="ExternalInput"),
        "w_q1": nc.dram_tensor("w_q1", [128, NCT, 2, 128], FP8,
                               kind="ExternalInput"),
        "w_k0": nc.dram_tensor("w_k0", [128, NCT, 2, 128], FP8,
                               kind="ExternalInput"),
        "w_k1": nc.dram_tensor("w_k1", [128, NCT, 2, 128], FP8,
                               kind="ExternalInput"),
        "w_v": nc.dram_tensor("w_v", [128, NCT, 2, 256], FP8,
                              kind="ExternalInput"),
        "wp_eff": nc.dram_tensor("wp_eff", [128, 2, C], BF16,
                                 kind="ExternalInput"),
        "masks": nc.dram_tensor("masks", [128, 2, 128], BF16,
                                kind="ExternalInput"),
    }
    outT = nc.dram_tensor("outT", [C, T], BF16, kind="ExternalOutput")
    if _DEBUG:
        t_in["q8_dbg"] = nc.dram_tensor("q8_dbg", [128, 2, T], FP8,
                                        kind="ExternalOutput")
        t_in["khl_dbg"] = nc.dram_tensor("khl_dbg", [128, 2, 2, T], FP8,
                                         kind="ExternalOutput")
        t_in["v1_dbg"] = nc.dram_tensor("v1_dbg", [128, HPC, KT, HD + 1],
                                        BF16, kind="ExternalOutput")
        t_in["yn_dbg"] = nc.dram_tensor("yn_dbg", [128, 2, T], BF16,
                                        kind="ExternalOutput")
        t_in["ys_dbg"] = nc.dram_tensor("ys_dbg", [128, 2, 8, HD], BF16,
                                        kind="ExternalOutput")
        t_in["rc_dbg"] = nc.dram_tensor("rc_dbg", [128, 2, 8], F32,
                                        kind="ExternalOutput")
    return t_in, outT


def _build(reps: int = 1):
    nc = bacc.Bacc("TRN2", target_bir_lowering=False, debug=False)
    t_in, outT = _declare_io(nc)
    with tile.TileContext(nc) as tc:
        with ExitStack() as ctx:
            _emit(ctx, tc, t_in, outT, reps=reps)
    nc.compile()
    return nc


def _fp8_split(a: np.ndarray):
    """Return (hi, lo) fp8e4m3 pair with hi + lo ~= a."""
    f8 = ml_dtypes.float8_e4m3
    hi = a.astype(f8)
    lo = (a - hi.astype(np.float32)).astype(f8)
    return hi, lo


def _make_in_maps(inputs: dict) -> list:
    f32 = np.float32
    f8 = ml_dtypes.float8_e4m3
    x = np.asarray(inputs["x"], f32)                     # [B, T, C]
    w_attn = np.asarray(inputs["w_attn"], f32)
    la_attn = np.ascontiguousarray(np.asarray(inputs["la_attn"], f32))
    lb_attn = np.asarray(inputs["lb_attn"], f32)
    w_proj = np.asarray(inputs["w_proj"], f32)
    la_proj = np.asarray(inputs["la_proj"], f32)
    lb_proj = np.asarray(inputs["lb_proj"], f32)

    # fold LoRA into effective weights on the host (input preprocessing)
    Wq = w_attn + 0.5 * lb_attn @ la_attn                # [3C, C]
    Wp = w_proj + 0.5 * lb_proj @ la_proj                # [C, C]

    # x: [128, tc8, hl, ct, tok] fp8, scaled by XS
    xq_b = []
    for b in range(B):
        xT = np.ascontiguousarray(x[b].T) * XS           # [C, T]
        xr = xT.reshape(NCT, 128, T)                     # [ct, p, t]
        hi, lo = _fp8_split(xr)
        arr = np.empty((128, NTC, 2, NCT, TCH), f8)
        for tc8 in range(NTC):
            sl = slice(tc8 * TCH, (tc8 + 1) * TCH)
            arr[:, tc8, 0] = hi[:, :, sl].transpose(1, 0, 2)
            arr[:, tc8, 1] = lo[:, :, sl].transpose(1, 0, 2)
        xq_b.append(arr)

    k_idx = np.arange(128)[:, None]
    q_idx = np.arange(128)[None, :]
    masks = np.zeros((128, 2, 128), ml_dtypes.bfloat16)
    masks[:, 0, :] = (q_idx > k_idx)         # A[j, k] = 1 iff k > j
    masks[:, 1, :] = MASK_VAL * (q_idx == k_idx)

    in_maps = []
    for core in range(NCORES):
        b, g = core // 4, core % 4
        ch0 = g * CH
        rows = np.r_[ch0:ch0 + CH, C + ch0:C + ch0 + CH,
                     2 * C + ch0:2 * C + ch0 + CH]
        # [p, ct, hl, r] = fp8 split of (WS * Wq).T[ct*128+p, r], per row block
        wq_s = np.ascontiguousarray(
            (WS * Wq[rows]).T.reshape(NCT, 128, NQR).transpose(1, 0, 2))
        w_hi, w_lo = _fp8_split(wq_s)                    # [p, ct, 768]
        whl = np.stack([w_hi, w_lo], axis=2)             # [p, ct, 2, 768]
        # [p, cht, c] = Wp.T[ch0+cht*128+p, c]
        wp_eff = np.ascontiguousarray(
            Wp[:, ch0:ch0 + CH].T.reshape(2, 128, C).transpose(1, 0, 2)
        ).astype(ml_dtypes.bfloat16)
        in_maps.append({
            "xq": xq_b[b],
            "w_q0": np.ascontiguousarray(whl[:, :, :, 0:128]),
            "w_q1": np.ascontiguousarray(whl[:, :, :, 128:256]),
            "w_k0": np.ascontiguousarray(whl[:, :, :, 256:384]),
            "w_k1": np.ascontiguousarray(whl[:, :, :, 384:512]),
            "w_v": np.ascontiguousarray(whl[:, :, :, 512:768]),
            "wp_eff": wp_eff,
            "masks": masks,
        })
    return in_maps


def _execute(inputs: dict, trace: bool = False):
    if "nc" not in _CACHE:
        _CACHE["nc"] = _build()
    nc = _CACHE["nc"]
    in_maps = _make_in_maps(inputs)
    res = run_bass_kernel_spmd(nc, in_maps, core_ids=list(range(NCORES)),
                               trace=trace)
    out = np.empty((B, T, C), np.float32)
    for b in range(B):
        acc = np.zeros((C, T), np.float32)
        for g in range(4):
            acc += np.asarray(res.results[b * 4 + g]["outT"], dtype=np.float32)
        out[b] = acc.T
    return out, res


def kernel(**inputs) -> np.ndarray:
    out, _ = _execute(inputs, trace=False)
    return out


# revision 54
# speedup vs baseline: 1.1109x; 1.0228x over previous
"""Trainium2 Bass kernel for a causal self-attention block with LoRA adapters.

Model (B=2, T=2048, C=1024, H=16 heads, hd=64, LoRA r=32, scale 0.5):
    qkv = x @ w_attn.T + 0.5*(x @ la_attn.T) @ lb_attn.T      (biases are 0)
    y   = causal_softmax_attention(q, k, v)
    out = y @ w_proj.T + 0.5*(y @ la_proj.T) @ lb_proj.T

Sharding: 8 cores = 2 batches x 4 head-groups. Core c owns batch c//4 and
heads 4*(c%4)..4*(c%4)+3: column-split c_attn (its 768 q/k/v rows over its
batch's 2048 tokens), full attention for its 4 heads, row-split c_proj
producing a 4-way partial [C, T]; the host sums 4 partials per batch.

Device algorithm per core (fp32 PSUM everywhere):
  - LoRA folded into effective weights on the host.  The big GEMMs use
    fp8e4m3 DoubleRow matmuls (0.5 PE rows/cycle, 256-deep contraction):
    * qkv: x and W shipped as (hi, lo) fp8 pairs (x_s = 4x, W_s = 128W);
      3-term product Whi*(xhi+xlo) + Wlo*xhi.  The hi/lo pair rides dim1 of
      one DoubleRow matmul with the other operand broadcast (0-stride), so
      the 3 terms cost 6 bf16-equivalent passes instead of 8.
    * S = q.k: k is stored as an (hi, lo) fp8 pair; one DoubleRow matmul
      per key-tile contracts [k_hi; k_lo] x [q; q] (broadcast) -> k at full
      precision, only q carries fp8 rounding.  Half the bf16 PE time.
    * causal mask: folded into the S accumulation group as one extra bf16
      matmul  st[:,diag] += A^T B  (A = strict upper ones, B = -16384*I),
      so exp() gives exact zeros and no per-block mask multiply is needed.
    * AV and c_proj stay bf16 (P cannot be quantized to fp8 cheaply).
  - attention per (j2: 1024-wide q chunk, h): S^T[k, q] blocks into PSUM,
    P = exp(S * 2^-7) on ScalarE; AV in [q, d] orientation with a 0.25
    column appended to v so yp[:,64] = den/4 and ys = yp * (4/den) = 4y.
  - normalize while tokens are on partitions (DVE reciprocal + 8 scaled
    PSUM->SBUF copies), transpose back to [ch, tok] via XBAR DMA transpose
    (mid-stream) or PE identity matmul (latency-critical tail).
  - outT_partial = Wp^T @ yn per 128-channel tile, *0.25 fused into the
    PSUM->SBUF copies.  Schedule: qkv/proj chunks drain into PE gaps in
    priority bands so neither PE nor the ScalarE exp stream starves.
Output: bf16 partial [C, T] per core; host sums 4 partials per batch in f32.
"""

from contextlib import ExitStack

import numpy as np
import ml_dtypes

import concourse.bass as bass
import concourse.tile as tile
from concourse import bacc, mybir
from concourse.bass_utils import run_bass_kernel_spmd

F32 = mybir.dt.float32
BF16 = mybir.dt.bfloat16
FP8 = mybir.dt.float8e4
AF = mybir.ActivationFunctionType
ALU = mybir.AluOpType
DR = mybir.MatmulPerfMode.DoubleRow

B, T, C, H, R = 2, 2048, 1024, 16, 32
HD = C // H              # 64
NCORES = 8
HPC = 4                  # heads per core
CH = HPC * HD            # 256 per-core channels
NCT = C // 128           # 8 contraction tiles
NQR = 3 * CH             # 768 qkv rows per core
NMT = 2 * CH // 128      # 4 q+k partition tiles
KT = T // 128            # 16 key tiles
QW = 1024                # q chunk width
TCH = 512                # token chunk for qkv/proj
NTC = T // TCH           # 4

XS = 4.0                 # host scale on x
WS = 128.0               # host scale on w_attn
QKV_SCALE = 2.0 ** -7    # psum (= 512 * raw) -> 4 * raw for q/k fp8
V_SCALE = 2.0 ** -9      # psum -> raw for v (bf16)
EXP_SCALE = 2.0 ** -7    # S_psum = 16 * S_raw; want exp(S_raw / 8)
ONES_VAL = 0.25          # v denominator column -> ys = 4 * y
PROJ_SCALE = 0.25        # proj psum (= 4 * out) -> out
MASK_VAL = -16384.0      # masked S entries (exp -> 0)

_CACHE: dict = {}
_PHASE_MARKS: list = []
_ABLATE: set = set()
_DEBUG = False


def _mark(nc, name):
    _PHASE_MARKS.append((name, nc.next_id()))


def _emit(ctx: ExitStack, tc: tile.TileContext, t_in: dict, outT, reps: int = 1):
    nc = tc.nc
    _PHASE_MARKS.clear()
    _mark(nc, "setup")

    singles = ctx.enter_context(tc.tile_pool(name="singles", bufs=1))
    psS = ctx.enter_context(tc.tile_pool(name="psS", bufs=2, space=bass.MemorySpace.PSUM))
    psY = ctx.enter_context(tc.tile_pool(name="psY", bufs=1, space=bass.MemorySpace.PSUM))
    psA = ctx.enter_context(tc.tile_pool(name="psA", bufs=2, space=bass.MemorySpace.PSUM))
    ptp = ctx.enter_context(tc.tile_pool(name="ptp", bufs=24))
    ysp = ctx.enter_context(tc.tile_pool(name="ysp", bufs=8))
    rcp = ctx.enter_context(tc.tile_pool(name="rcp", bufs=8))
    outp = ctx.enter_context(tc.tile_pool(name="outp", bufs=8))

    # ---------- constants / weights to SBUF ----------
    # x ships as fp8 (hi, lo) pairs laid out [128, ct, hl, tok] per 512-token
    # chunk; weights as fp8 hi + lo planes. Queues: scalar (ACT) carries
    # weights, sync (SP) the x head, gpsimd (Pool SWDGE) the x tail.
    # x per 512-token chunk, hi and lo planes in SEPARATE tiles: DMA-write
    # -> compute-read dependencies are tile-granular, so the hi-only T1+T3
    # matmuls must not share a tile with the later-arriving lo plane.
    xh = [singles.tile([128, NCT, TCH], FP8, name=f"xh{i}")
          for i in range(NTC)]
    xl = [singles.tile([128, NCT, TCH], FP8, name=f"xl{i}")
          for i in range(NTC)]
    # weights per row-block, (hi, lo) interleaved on dim2: one DoubleRow
    # matmul per c computes Whi*xhi + Wlo*xhi with the pair as stationary;
    # the Whi*xlo correction pairs c-planes (stride-2 APs into the same
    # tiles).
    wq_sb = [singles.tile([128, NCT, 2, 128], FP8, name=f"wq{m}")
             for m in range(2)]                    # q rows per head-pair
    wk_sb = [singles.tile([128, NCT, 2, 128], FP8, name=f"wk{m}")
             for m in range(2)]                    # k rows per head-pair
    wv_sb = singles.tile([128, NCT, 2, 256], FP8)  # v rows 512:768
    wp_sb = singles.tile([128, 2, C], BF16)
    mask_sb = singles.tile([128, 2, 128], BF16)  # [:,0]=A ones, [:,1]=B diag

    _mark(nc, "xload")

    def xload(tc8, hl, queue):
        queue.dma_start((xh if hl == 0 else xl)[tc8][:],
                        t_in["xq"][:, tc8, hl])

    # Critical-order loading: the first attention unit needs (in order)
    # wk/wq rows for heads 0/1, x chunk0 hi, then the lo planes.  sync and
    # scalar HWDGE queues interleave on the single DMA-engine pool, so the
    # emission order here IS the landing order.  The x tail (chunks 2-3)
    # goes on the Pool SWDGE queue but is emitted later (at attn0) so its
    # transfers don't steal DMA-engine slots from the critical pieces.
    nc.scalar.dma_start(mask_sb[:], t_in["masks"][:])
    nc.scalar.dma_start(wk_sb[0][:], t_in["w_k0"][:])
    if "xload" not in _ABLATE:
        xload(0, 0, nc.sync)
        xload(0, 1, nc.sync)
    nc.scalar.dma_start(wq_sb[0][:], t_in["w_q0"][:])
    nc.scalar.dma_start(wk_sb[1][:], t_in["w_k1"][:])
    if "xload" not in _ABLATE:
        xload(1, 0, nc.sync)
    nc.scalar.dma_start(wq_sb[1][:], t_in["w_q1"][:])
    if "xload" not in _ABLATE:
        xload(1, 1, nc.sync)
    nc.scalar.dma_start(wv_sb[:], t_in["w_v"][:])
    nc.scalar.dma_start(wp_sb[:], t_in["wp_eff"][:])

    def xload_tail():
        if "xload" not in _ABLATE:
            # The Pool queue is otherwise empty, so its SWDGE loads would
            # fire at t=0 and steal DMA-engine slots from the critical
            # head pieces.  Gate each tail DMA behind the last critical x
            # piece by first writing its dest tile with a tiny Pool copy
            # that reads xl[1] (write->write ordering is tile-granular).
            for tc8 in range(2, 4):
                for hl in range(2):
                    dst = (xh if hl == 0 else xl)[tc8]
                    nc.gpsimd.tensor_copy(dst[0:1, 0, 0:8],
                                          xl[1][0:1, 0, 0:8])
                    xload(tc8, hl, nc.gpsimd)

    # PE p-state warmup: the clock ramps only while the engine is
    # continuously busy (>3us to reach 2.4GHz), so spin defined-value
    # matmuls into a scratch PSUM bank while the first x/w DMAs are in
    # flight.  Sized to end just as the first real chunk's inputs land.
    warm_sb = singles.tile([128, TCH], BF16)
    nc.vector.memset(warm_sb[:], 0.0)

    for _rep in range(reps):
        q8 = singles.tile([128, 2, T], FP8)        # q as fp8, 4*q_raw
        khl = singles.tile([128, 2, 2, T], FP8)    # k (hi, lo) pairs
        v1 = singles.tile([128, HPC, KT, HD + 1], BF16)
        nc.vector.memset(v1[:, :, :, HD:HD + 1], ONES_VAL)
        warm_ps = psY.tile([128, 8, 128], F32, tag="yp", name="warm")
        for wi in range(10):
            nc.tensor.matmul(warm_ps[:, 0:4, :], warm_sb[:, 0:128],
                             warm_sb[:], start=True, stop=True)
        # yn.T = 4*y per channel tile, split per (j2, cht) so a proj chunk
        # only depends on the dphase DMA writes of its own token half
        # (DMA-write -> read deps are tile-granular).
        yn = {(j2, cht): singles.tile([128, QW], BF16, name=f"yn{j2}{cht}")
              for j2 in range(2) for cht in range(2)}
        if "attn" in _ABLATE:
            for t in yn.values():
                nc.vector.memset(t[:], 1.0)

        def qkv_matmuls(ps, tc8, wt, rows, tok=slice(0, TCH),
                        x_station=False):
            """12 DoubleRow matmuls: (Whi+Wlo)*xhi + Whi*xlo.

            x-stationary (v) flavor uses c-pair form throughout so the hi
            and lo planes stay in separate tiles:
              (xhi[c],xhi[c+1])x(Whi[c],Whi[c+1]) + same x(Wlo..) + xlo x Whi
            """
            if x_station:
                for t13 in range(2):   # 0: xhi*Whi pairs, 1: xhi*Wlo pairs
                    for cp in range(NCT // 2):
                        cs = slice(2 * cp, 2 * cp + 2)
                        nc.tensor.matmul(
                            ps[:], xh[tc8][:, cs, tok], wt[:, cs, t13, rows],
                            start=(t13 == 0 and cp == 0), stop=False,
                            perf_mode=DR)
                for cp in range(NCT // 2):
                    cs = slice(2 * cp, 2 * cp + 2)
                    nc.tensor.matmul(
                        ps[:], xl[tc8][:, cs, tok], wt[:, cs, 0, rows],
                        start=False, stop=(cp == NCT // 2 - 1), perf_mode=DR)
                return
            for c in range(NCT):
                # T1+T3: stationary (Whi[c], Wlo[c]) pair x broadcast xhi[c]
                nc.tensor.matmul(
                    ps[:], wt[:, c, :, rows],
                    xh[tc8][:, c, tok].unsqueeze(1).broadcast_to(
                        [128, 2, TCH]),
                    start=(c == 0), stop=False, perf_mode=DR)
            for cp in range(NCT // 2):
                # T2: (Whi[c], Whi[c+1]) pair x (xlo[c], xlo[c+1]) pair
                cs = slice(2 * cp, 2 * cp + 2)
                nc.tensor.matmul(ps[:], wt[:, cs, 0, rows],
                                 xl[tc8][:, cs, tok], start=False,
                                 stop=(cp == NCT // 2 - 1), perf_mode=DR)

        def emit_qk_chunk(tc8, mt, eng="dve"):
            sl = slice(tc8 * TCH, (tc8 + 1) * TCH)
            ps = psA.tile([128, TCH], F32, tag="a", name=f"qk{tc8}_{mt}")
            wt = (wq_sb[mt] if mt < 2 else wk_sb[mt - 2])
            qkv_matmuls(ps, tc8, wt, slice(0, 128))
            if mt < 2:  # q -> single fp8
                if eng == "act":
                    nc.scalar.activation(q8[:, mt, sl], ps[:], AF.Copy,
                                         scale=QKV_SCALE)
                else:
                    nc.vector.tensor_scalar(q8[:, mt, sl], ps[:],
                                            QKV_SCALE, None, ALU.mult)
            else:       # k -> (hi, lo) fp8 pair
                kh = khl[:, mt - 2, 0, sl]
                if eng == "act":
                    nc.scalar.activation(kh, ps[:], AF.Copy, scale=QKV_SCALE)
                else:
                    nc.vector.tensor_scalar(kh, ps[:], QKV_SCALE, None,
                                            ALU.mult)
                nc.vector.scalar_tensor_tensor(
                    khl[:, mt - 2, 1, sl], ps[:], QKV_SCALE, kh,
                    ALU.mult, ALU.subtract)

        def emit_v_chunk(kt):
            ps = psA.tile([128, CH], F32, tag="a", name=f"v{kt}",
                          padded_shape=[128, 512])
            qkv_matmuls(ps, kt // 4, wv_sb, slice(0, CH),
                        tok=slice((kt % 4) * 128, (kt % 4 + 1) * 128),
                        x_station=True)
            nc.vector.tensor_scalar(
                v1[:, :, kt, 0:HD],
                ps[:].rearrange("p (h d) -> p h d", h=HPC),
                V_SCALE, None, ALU.mult)

        def emit_proj_single(mt, tc8, eng="dve", dmaq="sync", pool=None):
            sl = slice(tc8 * TCH, (tc8 + 1) * TCH)
            po = (pool or psA).tile([128, TCH], F32,
                                    tag="a" if pool is None else "st",
                                    name=f"po{mt}_{tc8}")
            lsl = slice((tc8 % 2) * TCH, (tc8 % 2 + 1) * TCH)
            for cht in range(2):
                nc.tensor.matmul(po[:],
                                 wp_sb[:, cht, mt * 128:(mt + 1) * 128],
                                 yn[(tc8 // 2, cht)][:, lsl],
                                 start=(cht == 0), stop=(cht == 1))
            ot = outp.tile([128, TCH], BF16, tag="ots")
            if eng == "act":
                nc.scalar.activation(ot[:], po[:], AF.Copy, scale=PROJ_SCALE)
            elif eng == "pool":
                nc.gpsimd.tensor_scalar(ot[:], po[:], PROJ_SCALE, None,
                                        ALU.mult)
            else:
                nc.vector.tensor_scalar(ot[:], po[:], PROJ_SCALE, None,
                                        ALU.mult)
            getattr(nc, dmaq).dma_start(outT[mt * 128:(mt + 1) * 128, sl],
                                        ot[:])

        def emit_proj_pair(mt, pair, engs=("dve", "dve"), dmaq="gpsimd"):
            ot = outp.tile([128, 2, TCH], BF16, tag="ot")
            for half in range(2):
                tc8 = pair * 2 + half
                sl = slice(tc8 * TCH, (tc8 + 1) * TCH)
                po = psA.tile([128, TCH], F32, tag="a", name=f"po{mt}_{tc8}")
                lsl = slice((tc8 % 2) * TCH, (tc8 % 2 + 1) * TCH)
                for cht in range(2):
                    nc.tensor.matmul(po[:],
                                     wp_sb[:, cht, mt * 128:(mt + 1) * 128],
                                     yn[(tc8 // 2, cht)][:, lsl],
                                     start=(cht == 0), stop=(cht == 1))
                if engs[half] == "act":
                    nc.scalar.activation(ot[:, half], po[:], AF.Copy,
                                         scale=PROJ_SCALE)
                elif engs[half] == "pool":
                    nc.gpsimd.tensor_scalar(ot[:, half], po[:], PROJ_SCALE,
                                            None, ALU.mult)
                else:
                    nc.vector.tensor_scalar(ot[:, half], po[:], PROJ_SCALE,
                                            None, ALU.mult)
            getattr(nc, dmaq).dma_start(
                outT[mt * 128:(mt + 1) * 128,
                     pair * 2 * TCH:(pair * 2 + 2) * TCH], ot[:])

        fillers: list = []

        def drain(n):
            # qkv fillers gate future exps: keep them at normal priority.
            # proj fillers are pure sinks: push them to low priority.
            save = tc.cur_priority
            try:
                for _ in range(min(n, len(fillers))):
                    kind, fn = fillers.pop(0)
                    tc.cur_priority = save + {"gate": 8000, "v": 12000,
                                              "sink": 16000}[kind]
                    fn()
            finally:
                tc.cur_priority = save
            return

        ys_tiles: dict = {}

        def emit_attn_head(j2, h, fill_every=2, fill_at=None,
                           split_exp=False):
            kmt = h // 2
            qmt = h // 2
            nkt = 8 * j2 + 8
            q0 = j2 * QW
            yp = psY.tile([128, 8, 128], F32, tag="yp", name=f"yp{j2}_{h}")
            rc = rcp.tile([128, 8], F32, tag="rc", name=f"rc{j2}_{h}")
            p0 = (h % 2) * 64
            if h % 2 == 0:
                ysc = ysp.tile([128, 8, 128], BF16, tag="ys",
                               name=f"ys{j2}_{h // 2}")
                ys_tiles[(j2, h // 2)] = ysc
            else:
                ysc = ys_tiles[(j2, h // 2)]
            ys = ysc[:, :, p0:p0 + HD]
            for kt in range(nkt):
                lead = (kt // 8 == j2)
                cs = 128 * (kt % 8) if lead else 0
                kp = (h % 2) * 64
                k_lhs = khl[kp:kp + 64, kmt, :, kt * 128:(kt + 1) * 128]
                st = psS.tile([128, QW], F32, tag="st", name=f"st{j2}_{h}_{kt}")
                ranges = ((cs, 512), (512, QW)) if cs < 512 else ((cs, QW),)
                for lo, hi in ranges:
                    # each range opens its own PSUM bank group (start=True);
                    # a start=False matmul here would accumulate onto stale
                    # bank contents from the previous st tile use.
                    q_rhs = q8[kp:kp + 64, qmt, q0 + lo:q0 + hi] \
                        .unsqueeze(1).broadcast_to([64, 2, hi - lo])
                    nc.tensor.matmul(st[:, lo:hi], k_lhs, q_rhs,
                                     start=True, stop=True, perf_mode=DR)
                if lead:
                    # causal mask: st[:, diag] += A^T B (-16384 above diag)
                    nc.tensor.matmul(st[:, cs:cs + 128], mask_sb[:, 0, :],
                                     mask_sb[:, 1, :], start=False, stop=True,
                                     skip_group_check=True)
                pt = ptp.tile([128, QW], BF16, tag="pt")
                if split_exp and cs < 512:
                    # halve the first unit's exps so the stream starts as
                    # soon as the first q8 chunk lands
                    nc.scalar.activation(pt[:, cs:512], st[:, cs:512],
                                         AF.Exp, scale=EXP_SCALE)
                    nc.scalar.activation(pt[:, 512:], st[:, 512:], AF.Exp,
                                         scale=EXP_SCALE)
                else:
                    nc.scalar.activation(pt[:, cs:], st[:, cs:], AF.Exp,
                                         scale=EXP_SCALE)
                # PSUM zero regions are bank-wide (2KB): only one accumulation
                # group per bank. Open each bank once (j=0/j=4 at kt=0); the
                # bank-wide pending-zero gives the other subtiles their
                # initial zeroing; close with the bank's last accumulation.
                j0 = max(0, kt - 8 * j2)
                for j in range(j0, 8):
                    nc.tensor.matmul(yp[:, j, 0:HD + 1],
                                     pt[:, j * 128:(j + 1) * 128],
                                     v1[:, h, kt, :],
                                     start=(kt == 0 and j % 4 == 0),
                                     stop=(j % 4 == 3 and kt == 8 * j2 + j))
                if kt == 8 * j2 + 3:
                    # bank 0 (subtiles 0-3) just closed: normalize its half
                    # now, 4 k-tiles before the unit ends
                    nc.vector.reciprocal(rc[:, 0:4], yp[:, 0:4, HD])
                    for j in range(4):
                        nc.vector.tensor_scalar(ys[:, j, :], yp[:, j, 0:HD],
                                                rc[:, j:j + 1], None, ALU.mult)
                if fill_at is not None:
                    if kt in fill_at:
                        drain(1)
                elif (kt + 1) % fill_every == 0:
                    drain(1)
            # bank 1 half (the last head's copies split across DVE/ACT to
            # shorten the post-stream tail)
            nc.vector.reciprocal(rc[:, 4:8], yp[:, 4:8, HD])
            tail_head = (j2 == 1 and h == HPC - 1)
            for j in range(4, 8):
                if tail_head and j % 2 == 1:
                    nc.scalar.activation(ys[:, j, :], yp[:, j, 0:HD],
                                         AF.Copy, scale=rc[:, j:j + 1])
                else:
                    nc.vector.tensor_scalar(ys[:, j, :], yp[:, j, 0:HD],
                                            rc[:, j:j + 1], None, ALU.mult)
            if _DEBUG and h == 0:
                nc.sync.dma_start(t_in["ys_dbg"][:, j2], ys[:])
                nc.sync.dma_start(t_in["rc_dbg"][:, j2], rc[:])

        def emit_dphase_half(j2, cht, half, ysc, eng="dve"):
            # transpose back: yn[ch, tok] = ys[q, ch].T
            # mid-stream phases use the XBAR DMA transpose on the idle SP
            # queue (frees PE + DVE); eng="pe" keeps the identity-matmul path
            # for the latency-critical final phase.
            if eng == "dma":
                for jj in range(4):
                    j = half * 4 + jj
                    t0 = j * 128
                    nc.sync.dma_start(yn[(j2, cht)][:, t0:t0 + 128],
                                      ysc[:, j, :], transpose=True)
                return
            ys_pair = [ysc[:, :, 0:HD], ysc[:, :, HD:2 * HD]]
            if True:
                dout = psA.tile([128, 512], F32, tag="a",
                                name=f"do{j2}_{cht}_{half}")
                for hh in range(2):
                    for jj in range(4):
                        j = half * 4 + jj
                        nc.tensor.matmul(dout[hh * 64:(hh + 1) * 64,
                                              jj * 128:(jj + 1) * 128],
                                         ys_pair[hh][:, j, :],
                                         mask_sb[:, 1, :],
                                         start=True, stop=True)
                t0 = half * 512
                ynt = yn[(j2, cht)]
                if eng == "act":
                    nc.scalar.activation(ynt[:, t0:t0 + 512], dout[:],
                                         AF.Copy, scale=1.0 / MASK_VAL)
                elif eng == "split":
                    nc.vector.tensor_scalar(ynt[:, t0:t0 + 256],
                                            dout[:, 0:256], 1.0 / MASK_VAL,
                                            None, ALU.mult)
                    nc.scalar.activation(ynt[:, t0 + 256:t0 + 512],
                                         dout[:, 256:512], AF.Copy,
                                         scale=1.0 / MASK_VAL)
                else:
                    nc.vector.tensor_scalar(ynt[:, t0:t0 + 512],
                                            dout[:], 1.0 / MASK_VAL, None,
                                            ALU.mult)

        def emit_dphase(j2, cht, engs=("dve", "dve")):
            ysc = ys_tiles.pop((j2, cht))
            for half in range(2):
                emit_dphase_half(j2, cht, half, ysc, engs[half])

        # ---------- schedule ----------
        _mark(nc, "qkv0")
        for tc8 in range(2):
            for mt in (2, 0):   # k first: the k hi+lo copies gate the
                emit_qk_chunk(tc8, mt, eng="act")  # first S matmul
        save_p = tc.cur_priority
        tc.cur_priority = save_p + 12000
        for kt in range(4):
            emit_v_chunk(kt)
        tc.cur_priority = save_p

        if "attn" not in _ABLATE:
            # All four j2=0 units run early so dphase(0,*) completes and the
            # 16 proj chunks for tokens 0:1024 become PE filler for the
            # three final j2=1 units; fillers sized per unit.
            def qkf(tc8, mt):
                fillers.append(("gate", lambda: emit_qk_chunk(tc8, mt)))

            def vf(kt):
                fillers.append(("v", lambda: emit_v_chunk(kt)))

            _mark(nc, "attn0")
            xload_tail()
            qkf(2, 0); qkf(3, 0)
            for kt in range(4, 8):
                vf(kt)
            qkf(0, 1); qkf(0, 3)
            with tc.high_priority(offset=4000):
                emit_attn_head(0, 0, fill_at=set(range(8)), split_exp=True)
            qkf(2, 2); qkf(3, 2); qkf(1, 1); qkf(1, 3)
            with tc.high_priority(offset=4000):
                emit_attn_head(0, 1, fill_at={0, 1, 2, 3}, split_exp=True)
            emit_dphase(0, 0, engs=("dve", "dve"))
            for kt in range(8, 16):
                vf(kt)
            with tc.high_priority(offset=4000):
                emit_attn_head(1, 0, fill_at=set(range(8)))
            qkf(2, 1); qkf(3, 1); qkf(2, 3); qkf(3, 3)
            with tc.high_priority(offset=4000):
                emit_attn_head(0, 2, fill_at={1, 3, 5, 7})
            with tc.high_priority(offset=4000):
                emit_attn_head(0, 3, fill_at={1, 3, 5, 7})
            emit_dphase(0, 1, engs=("dve", "dve"))
            _mark(nc, "attn1")
            for mt in range(NCT):
                fillers.append(("sink", lambda mt=mt: emit_proj_pair(mt, 0)))
            with tc.high_priority(offset=4000):
                emit_attn_head(1, 1, fill_at={3, 7, 11, 15})
            emit_dphase(1, 0, engs=("dma", "dma"))
            with tc.high_priority(offset=4000):
                emit_attn_head(1, 2, fill_at={1, 5, 9, 13})
            with tc.high_priority(offset=4000):
                emit_attn_head(1, 3, fill_at={1, 5, 9, 13})
            _mark(nc, "dphase1")
            drain(len(fillers))
            ysc_t = ys_tiles.pop((1, 1))
            with tc.high_priority(offset=4000):
                emit_dphase_half(1, 1, 0, ysc_t, "dve")
            # tail proj pinned late via wait_until: the scheduler's internal
            # sim underestimates when yn(1,*) lands, and an early slot in
            # the in-order PE queue head-blocks the remaining attention
            # stream behind a not-yet-ready proj matmul.
            wu = float(__import__("os").environ.get("K_TAIL_WU", "0.087"))
            with tc.tile_wait_until(wu):
                for i, mt in enumerate(range(NCT)):
                    emit_proj_single(mt, 2, eng=("act", "dve")[i % 2],
                                     dmaq=("sync", "gpsimd")[i % 2],
                                     pool=(None, psS)[i % 2])
            with tc.high_priority(offset=4000):
                emit_dphase_half(1, 1, 1, ysc_t, "split")
            with tc.tile_wait_until(wu + 0.004):
                for i, mt in enumerate(range(NCT)):
                    emit_proj_single(mt, 3, eng=("dve", "act")[i % 2],
                                     dmaq=("gpsimd", "sync")[i % 2],
                                     pool=(None, psS)[i % 2])
        else:
            for tc8 in range(2):
                for mt in (1, 3):
                    emit_qk_chunk(tc8, mt)
            for tc8 in range(2, 4):
                for mt in range(NMT):
                    emit_qk_chunk(tc8, mt)
            for kt in range(8, 16):
                emit_v_chunk(kt)
            for mt in range(NCT):
                emit_proj_pair(mt, 0)

        _mark(nc, "projtail")
        if "proj" not in _ABLATE and "attn" in _ABLATE:
            engs = [("dve", "act"), ("act", "dve")]
            for mt in range(NCT):
                emit_proj_pair(mt, 1, engs=engs[mt % 2], dmaq="sync")

        if _DEBUG:
            nc.sync.dma_start(t_in["q8_dbg"][:], q8[:])
            nc.sync.dma_start(t_in["khl_dbg"][:], khl[:])
            nc.sync.dma_start(t_in["v1_dbg"][:], v1[:])
            for j2 in range(2):
                for cht in range(2):
                    nc.sync.dma_start(
                        t_in["yn_dbg"][:, cht, j2 * QW:(j2 + 1) * QW],
                        yn[(j2, cht)][:])


def _declare_io(nc):
    t_in = {
        # [128, tc8, hl, ct, tok] fp8 (hi/lo planes contiguous per chunk)
        "xq": nc.dram_tensor("xq", [128, NTC, 2, NCT, TCH], FP8,
                             kind="ExternalInput"),
        "w_q0": nc.dram_tensor("w_q0", [128, NCT, 2, 128], FP8,
                               kind="ExternalInput"),
        "w_q1": nc.dram_tensor("w_q1", [128, NCT, 2, 128], FP8,
                               kind="ExternalInput"),
        "w_k0": nc.dram_tensor("w_k0", [128, NCT, 2, 128], FP8,
                               kind="ExternalInput"),
        "w_k1": nc.dram_tensor("w_k1", [128, NCT, 2, 128], FP8,
                               kind="ExternalInput"),
        "w_v": nc.dram_tensor("w_v", [128, NCT, 2, 256], FP8,
                              kind="ExternalInput"),
        "wp_eff": nc.dram_tensor("wp_eff", [128, 2, C], BF16,
                                 kind="ExternalInput"),
        "masks": nc.dram_tensor("masks", [128, 2, 128], BF16,
                                kind="ExternalInput"),
    }
    outT = nc.dram_tensor("outT", [C, T], BF16, kind="ExternalOutput")
    if _DEBUG:
        t_in["q8_dbg"] = nc.dram_tensor("q8_dbg", [128, 2, T], FP8,
                                        kind="ExternalOutput")
        t_in["khl_dbg"] = nc.dram_tensor("khl_dbg", [128, 2, 2, T], FP8,
                                         kind="ExternalOutput")
        t_in["v1_dbg"] = nc.dram_tensor("v1_dbg", [128, HPC, KT, HD + 1],
                                        BF16, kind="ExternalOutput")
        t_in["yn_dbg"] = nc.dram_tensor("yn_dbg", [128, 2, T], BF16,
                                        kind="ExternalOutput")
        t_in["ys_dbg"] = nc.dram_tensor("ys_dbg", [128, 2, 8, HD], BF16,
                                        kind="ExternalOutput")
        t_in["rc_dbg"] = nc.dram_tensor("rc_dbg", [128, 2, 8], F32,
                                        kind="ExternalOutput")
    return t_in, outT


def _build(reps: int = 1):
    nc = bacc.Bacc("TRN2", target_bir_lowering=False, debug=False)
    t_in, outT = _declare_io(nc)
    with tile.TileContext(nc) as tc:
        with ExitStack() as ctx:
            _emit(ctx, tc, t_in, outT, reps=reps)
    nc.compile()
    return nc


def _fp8_split(a: np.ndarray):
    """Return (hi, lo) fp8e4m3 pair with hi + lo ~= a."""
    f8 = ml_dtypes.float8_e4m3
    hi = a.astype(f8)
    lo = (a - hi.astype(np.float32)).astype(f8)
    return hi, lo


def _make_in_maps(inputs: dict) -> list:
    f32 = np.float32
    f8 = ml_dtypes.float8_e4m3
    x = np.asarray(inputs["x"], f32)                     # [B, T, C]
    w_attn = np.asarray(inputs["w_attn"], f32)
    la_attn = np.ascontiguousarray(np.asarray(inputs["la_attn"], f32))
    lb_attn = np.asarray(inputs["lb_attn"], f32)
    w_proj = np.asarray(inputs["w_proj"], f32)
    la_proj = np.asarray(inputs["la_proj"], f32)
    lb_proj = np.asarray(inputs["lb_proj"], f32)

    # fold LoRA into effective weights on the host (input preprocessing)
    Wq = w_attn + 0.5 * lb_attn @ la_attn                # [3C, C]
    Wp = w_proj + 0.5 * lb_proj @ la_proj                # [C, C]

    # x: [128, tc8, hl, ct, tok] fp8, scaled by XS
    xq_b = []
    for b in range(B):
        xT = np.ascontiguousarray(x[b].T) * XS           # [C, T]
        xr = xT.reshape(NCT, 128, T)                     # [ct, p, t]
        hi, lo = _fp8_split(xr)
        arr = np.empty((128, NTC, 2, NCT, TCH), f8)
        for tc8 in range(NTC):
            sl = slice(tc8 * TCH, (tc8 + 1) * TCH)
            arr[:, tc8, 0] = hi[:, :, sl].transpose(1, 0, 2)
            arr[:, tc8, 1] = lo[:, :, sl].transpose(1, 0, 2)
        xq_b.append(arr)

    k_idx = np.arange(128)[:, None]
    q_idx = np.arange(128)[None, :]
    masks = np.zeros((128, 2, 128), ml_dtypes.bfloat16)
    masks[:, 0, :] = (q_idx > k_idx)         # A[j, k] = 1 iff k > j
    masks[:, 1, :] = MASK_VAL * (q_idx == k_idx)

    in_maps = []
    for core in range(NCORES):
        b, g = core // 4, core % 4
        ch0 = g * CH
        rows = np.r_[ch0:ch0 + CH, C + ch0:C + ch0 + CH,
                     2 * C + ch0:2 * C + ch0 + CH]
        # [p, ct, hl, r] = fp8 split of (WS * Wq).T[ct*128+p, r], per row block
        wq_s = np.ascontiguousarray(
            (WS * Wq[rows]).T.reshape(NCT, 128, NQR).transpose(1, 0, 2))
        w_hi, w_lo = _fp8_split(wq_s)                    # [p, ct, 768]
        whl = np.stack([w_hi, w_lo], axis=2)             # [p, ct, 2, 768]
        # [p, cht, c] = Wp.T[ch0+cht*128+p, c]
        wp_eff = np.ascontiguousarray(
            Wp[:, ch0:ch0 + CH].T.reshape(2, 128, C).transpose(1, 0, 2)
        ).astype(ml_dtypes.bfloat16)
        in_maps.append({
            "xq": xq_b[b],
            "w_q0": np.ascontiguousarray(whl[:, :, :, 0:128]),
            "w_q1": np.ascontiguousarray(whl[:, :, :, 128:256]),
            "w_k0": np.ascontiguousarray(whl[:, :, :, 256:384]),
            "w_k1": np.ascontiguousarray(whl[:, :, :, 384:512]),
            "w_v": np.ascontiguousarray(whl[:, :, :, 512:768]),
            "wp_eff": wp_eff,
            "masks": masks,
        })
    return in_maps


def _execute(inputs: dict, trace: bool = False):
    if "nc" not in _CACHE:
        _CACHE["nc"] = _build()
    nc = _CACHE["nc"]
    in_maps = _make_in_maps(inputs)
    res = run_bass_kernel_spmd(nc, in_maps, core_ids=list(range(NCORES)),
                               trace=trace)
    out = np.empty((B, T, C), np.float32)
    for b in range(B):
        acc = np.zeros((C, T), np.float32)
        for g in range(4):
            acc += np.asarray(res.results[b * 4 + g]["outT"], dtype=np.float32)
        out[b] = acc.T
    return out, res


def kernel(**inputs) -> np.ndarray:
    out, _ = _execute(inputs, trace=False)
    return out


# revision 65
# speedup vs baseline: 1.1421x; 1.0281x over previous
"""Trainium2 Bass kernel for a causal self-attention block with LoRA adapters.

Model (B=2, T=2048, C=1024, H=16 heads, hd=64, LoRA r=32, scale 0.5):
    qkv = x @ w_attn.T + 0.5*(x @ la_attn.T) @ lb_attn.T      (biases are 0)
    y   = causal_softmax_attention(q, k, v)
    out = y @ w_proj.T + 0.5*(y @ la_proj.T) @ lb_proj.T

Sharding: 8 cores = 2 batches x 4 head-groups. Core c owns batch c//4 and
heads 4*(c%4)..4*(c%4)+3: column-split c_attn (its 768 q/k/v rows over its
batch's 2048 tokens), full attention for its 4 heads, row-split c_proj
producing a 4-way partial [C, T]; the host sums 4 partials per batch.

Device algorithm per core (fp32 PSUM everywhere):
  - LoRA folded into effective weights on the host.  The big GEMMs use
    fp8e4m3 DoubleRow matmuls (0.5 PE rows/cycle, 256-deep contraction):
    * qkv: x and W shipped as (hi, lo) fp8 pairs (x_s = 4x, W_s = 128W);
      3-term product Whi*(xhi+xlo) + Wlo*xhi.  The hi/lo pair rides dim1 of
      one DoubleRow matmul with the other operand broadcast (0-stride), so
      the 3 terms cost 6 bf16-equivalent passes instead of 8.
    * S = q.k: k is stored as an (hi, lo) fp8 pair; one DoubleRow matmul
      per key-tile contracts [k_hi; k_lo] x [q; q] (broadcast) -> k at full
      precision, only q carries fp8 rounding.  Half the bf16 PE time.
    * causal mask: folded into the S accumulation group as one extra bf16
      matmul  st[:,diag] += A^T B  (A = strict upper ones, B = -16384*I),
      so exp() gives exact zeros and no per-block mask multiply is needed.
    * AV and c_proj stay bf16 (P cannot be quantized to fp8 cheaply).
  - attention per (j2: 1024-wide q chunk, h): S^T[k, q] blocks into PSUM,
    P = exp(S * 2^-7) on ScalarE; AV in [q, d] orientation with a 0.25
    column appended to v so yp[:,64] = den/4 and ys = yp * (4/den) = 4y.
  - normalize while tokens are on partitions (DVE reciprocal + 8 scaled
    PSUM->SBUF copies), transpose back to [ch, tok] via XBAR DMA transpose
    (mid-stream) or PE identity matmul (latency-critical tail).
  - outT_partial = Wp^T @ yn per 128-channel tile, *0.25 fused into the
    PSUM->SBUF copies.  Schedule: qkv/proj chunks drain into PE gaps in
    priority bands so neither PE nor the ScalarE exp stream starves.
Output: bf16 partial [C, T] per core; host sums 4 partials per batch in f32.
"""

from contextlib import ExitStack

import numpy as np
import ml_dtypes

import concourse.bass as bass
import concourse.tile as tile
from concourse import bacc, mybir
from concourse.bass_utils import run_bass_kernel_spmd

F32 = mybir.dt.float32
BF16 = mybir.dt.bfloat16
FP8 = mybir.dt.float8e4
AF = mybir.ActivationFunctionType
ALU = mybir.AluOpType
DR = mybir.MatmulPerfMode.DoubleRow

B, T, C, H, R = 2, 2048, 1024, 16, 32
HD = C // H              # 64
NCORES = 8
HPC = 4                  # heads per core
CH = HPC * HD            # 256 per-core channels
NCT = C // 128           # 8 contraction tiles
NQR = 3 * CH             # 768 qkv rows per core
NMT = 2 * CH // 128      # 4 q+k partition tiles
KT = T // 128            # 16 key tiles
QW = 1024                # q chunk width
TCH = 512                # token chunk for qkv/proj
NTC = T // TCH           # 4

XS = 4.0                 # host scale on x
WS = 128.0               # host scale on w_attn
QKV_SCALE = 2.0 ** -7    # psum (= 512 * raw) -> 4 * raw for q/k fp8
V_SCALE = 2.0 ** -9      # psum -> raw for v (bf16)
EXP_SCALE = 2.0 ** -7    # S_psum = 16 * S_raw; want exp(S_raw / 8)
ONES_VAL = 0.25          # v denominator column -> ys = 4 * y
PROJ_SCALE = 0.25        # proj psum (= 4 * out) -> out
MASK_VAL = -16384.0      # masked S entries (exp -> 0)

_CACHE: dict = {}
_PHASE_MARKS: list = []
_ABLATE: set = set()
_DEBUG = False


def _mark(nc, name):
    _PHASE_MARKS.append((name, nc.next_id()))


def _emit(ctx: ExitStack, tc: tile.TileContext, t_in: dict, outT, reps: int = 1):
    nc = tc.nc
    _PHASE_MARKS.clear()
    _mark(nc, "setup")

    singles = ctx.enter_context(tc.tile_pool(name="singles", bufs=1))
    psS = ctx.enter_context(tc.tile_pool(name="psS", bufs=2, space=bass.MemorySpace.PSUM))
    psY = ctx.enter_context(tc.tile_pool(name="psY", bufs=1, space=bass.MemorySpace.PSUM))
    psA = ctx.enter_context(tc.tile_pool(name="psA", bufs=2, space=bass.MemorySpace.PSUM))
    ptp = ctx.enter_context(tc.tile_pool(name="ptp", bufs=24))
    ysp = ctx.enter_context(tc.tile_pool(name="ysp", bufs=8))
    rcp = ctx.enter_context(tc.tile_pool(name="rcp", bufs=8))
    outp = ctx.enter_context(tc.tile_pool(name="outp", bufs=8))

    # ---------- constants / weights to SBUF ----------
    # x ships as fp8 (hi, lo) pairs laid out [128, ct, hl, tok] per 512-token
    # chunk; weights as fp8 hi + lo planes. Queues: scalar (ACT) carries
    # weights, sync (SP) the x head, gpsimd (Pool SWDGE) the x tail.
    # x per 512-token chunk, hi and lo planes in SEPARATE tiles: DMA-write
    # -> compute-read dependencies are tile-granular, so the hi-only T1+T3
    # matmuls must not share a tile with the later-arriving lo plane.
    xh = [singles.tile([128, NCT, TCH], FP8, name=f"xh{i}")
          for i in range(NTC)]
    xl = [singles.tile([128, NCT, TCH], FP8, name=f"xl{i}")
          for i in range(NTC)]
    # weights per row-block, (hi, lo) interleaved on dim2: one DoubleRow
    # matmul per c computes Whi*xhi + Wlo*xhi with the pair as stationary;
    # the Whi*xlo correction pairs c-planes (stride-2 APs into the same
    # tiles).
    wq_sb = [singles.tile([128, NCT, 2, 128], FP8, name=f"wq{m}")
             for m in range(2)]                    # q rows per head-pair
    wk_sb = [singles.tile([128, NCT, 2, 128], FP8, name=f"wk{m}")
             for m in range(2)]                    # k rows per head-pair
    wv_sb = singles.tile([128, NCT, 2, 256], FP8)  # v rows 512:768
    wp_sb = singles.tile([128, 2, C], BF16)
    mask_sb = singles.tile([128, 2, 128], BF16)  # [:,0]=A ones, [:,1]=B diag

    _mark(nc, "xload")

    def xload(tc8, hl, queue):
        queue.dma_start((xh if hl == 0 else xl)[tc8][:],
                        t_in["xq"][:, tc8, hl])

    # Critical-order loading: the first attention unit needs (in order)
    # wk/wq rows for heads 0/1, x chunk0 hi, then the lo planes.  sync and
    # scalar HWDGE queues interleave on the single DMA-engine pool, so the
    # emission order here IS the landing order.  The x tail (chunks 2-3)
    # goes on the Pool SWDGE queue but is emitted later (at attn0) so its
    # transfers don't steal DMA-engine slots from the critical pieces.
    nc.scalar.dma_start(mask_sb[:], t_in["masks"][:])
    nc.scalar.dma_start(wk_sb[0][:], t_in["w_k0"][:])
    if "xload" not in _ABLATE:
        xload(0, 0, nc.sync)
        xload(0, 1, nc.sync)
    nc.scalar.dma_start(wq_sb[0][:], t_in["w_q0"][:])
    nc.scalar.dma_start(wk_sb[1][:], t_in["w_k1"][:])
    if "xload" not in _ABLATE:
        xload(1, 0, nc.sync)
    nc.scalar.dma_start(wq_sb[1][:], t_in["w_q1"][:])
    if "xload" not in _ABLATE:
        xload(1, 1, nc.sync)
    nc.scalar.dma_start(wv_sb[:], t_in["w_v"][:])
    nc.scalar.dma_start(wp_sb[:], t_in["wp_eff"][:])

    def xload_tail():
        if "xload" not in _ABLATE:
            # The Pool queue is otherwise empty, so its SWDGE loads would
            # fire at t=0 and steal DMA-engine slots from the critical
            # head pieces.  Gate each tail DMA behind the last critical x
            # piece by first writing its dest tile with a tiny Pool copy
            # that reads xl[1] (write->write ordering is tile-granular).
            for tc8 in range(2, 4):
                for hl in range(2):
                    dst = (xh if hl == 0 else xl)[tc8]
                    nc.gpsimd.tensor_copy(dst[0:1, 0, 0:8],
                                          xl[1][0:1, 0, 0:8])
                    xload(tc8, hl, nc.gpsimd)

    # PE p-state warmup: the clock ramps only while the engine is
    # continuously busy (>3us to reach 2.4GHz), so spin defined-value
    # matmuls into a scratch PSUM bank while the first x/w DMAs are in
    # flight.  Sized to end just as the first real chunk's inputs land.
    warm_sb = singles.tile([128, TCH], BF16)
    nc.vector.memset(warm_sb[:], 0.0)

    for _rep in range(reps):
        q8 = singles.tile([128, 2, T], FP8)        # q as fp8, 4*q_raw
        khl = singles.tile([128, 2, 2, T], FP8)    # k (hi, lo) pairs
        v1 = singles.tile([128, HPC, KT, HD + 1], BF16)
        nc.vector.memset(v1[:, :, :, HD:HD + 1], ONES_VAL)
        warm_ps = psY.tile([128, 4, 128], F32, tag="ypA", name="warm")
        for wi in range(12):
            nc.tensor.matmul(warm_ps[:], warm_sb[:, 0:128],
                             warm_sb[:], start=True, stop=True)
        # yn.T = 4*y per channel tile, split per (j2, cht) so a proj chunk
        # only depends on the dphase DMA writes of its own token half
        # (DMA-write -> read deps are tile-granular).
        yn = {(j2, cht): singles.tile([128, QW], BF16, name=f"yn{j2}{cht}")
              for j2 in range(2) for cht in range(2)}
        if "attn" in _ABLATE:
            for t in yn.values():
                nc.vector.memset(t[:], 1.0)

        def qkv_matmuls(ps, tc8, wt, rows, tok=slice(0, TCH),
                        x_station=False):
            """12 DoubleRow matmuls: (Whi+Wlo)*xhi + Whi*xlo.

            x-stationary (v) flavor uses c-pair form throughout so the hi
            and lo planes stay in separate tiles:
              (xhi[c],xhi[c+1])x(Whi[c],Whi[c+1]) + same x(Wlo..) + xlo x Whi
            """
            if x_station:
                for t13 in range(2):   # 0: xhi*Whi pairs, 1: xhi*Wlo pairs
                    for cp in range(NCT // 2):
                        cs = slice(2 * cp, 2 * cp + 2)
                        nc.tensor.matmul(
                            ps[:], xh[tc8][:, cs, tok], wt[:, cs, t13, rows],
                            start=(t13 == 0 and cp == 0), stop=False,
                            perf_mode=DR)
                for cp in range(NCT // 2):
                    cs = slice(2 * cp, 2 * cp + 2)
                    nc.tensor.matmul(
                        ps[:], xl[tc8][:, cs, tok], wt[:, cs, 0, rows],
                        start=False, stop=(cp == NCT // 2 - 1), perf_mode=DR)
                return
            for c in range(NCT):
                # T1+T3: stationary (Whi[c], Wlo[c]) pair x broadcast xhi[c]
                nc.tensor.matmul(
                    ps[:], wt[:, c, :, rows],
                    xh[tc8][:, c, tok].unsqueeze(1).broadcast_to(
                        [128, 2, TCH]),
                    start=(c == 0), stop=False, perf_mode=DR)
            for cp in range(NCT // 2):
                # T2: (Whi[c], Whi[c+1]) pair x (xlo[c], xlo[c+1]) pair
                cs = slice(2 * cp, 2 * cp + 2)
                nc.tensor.matmul(ps[:], wt[:, cs, 0, rows],
                                 xl[tc8][:, cs, tok], start=False,
                                 stop=(cp == NCT // 2 - 1), perf_mode=DR)

        def emit_qk_chunk(tc8, mt, eng="dve"):
            sl = slice(tc8 * TCH, (tc8 + 1) * TCH)
            ps = psA.tile([128, TCH], F32, tag="a", name=f"qk{tc8}_{mt}")
            wt = (wq_sb[mt] if mt < 2 else wk_sb[mt - 2])
            qkv_matmuls(ps, tc8, wt, slice(0, 128))
            if mt < 2:  # q -> single fp8
                if eng == "act":
                    nc.scalar.activation(q8[:, mt, sl], ps[:], AF.Copy,
                                         scale=QKV_SCALE)
                else:
                    nc.vector.tensor_scalar(q8[:, mt, sl], ps[:],
                                            QKV_SCALE, None, ALU.mult)
            else:       # k -> (hi, lo) fp8 pair
                kh = khl[:, mt - 2, 0, sl]
                if eng == "act":
                    nc.scalar.activation(kh, ps[:], AF.Copy, scale=QKV_SCALE)
                else:
                    nc.vector.tensor_scalar(kh, ps[:], QKV_SCALE, None,
                                            ALU.mult)
                nc.vector.scalar_tensor_tensor(
                    khl[:, mt - 2, 1, sl], ps[:], QKV_SCALE, kh,
                    ALU.mult, ALU.subtract)

        def emit_v_chunk(kt):
            ps = psA.tile([128, CH], F32, tag="a", name=f"v{kt}",
                          padded_shape=[128, 512])
            qkv_matmuls(ps, kt // 4, wv_sb, slice(0, CH),
                        tok=slice((kt % 4) * 128, (kt % 4 + 1) * 128),
                        x_station=True)
            nc.vector.tensor_scalar(
                v1[:, :, kt, 0:HD],
                ps[:].rearrange("p (h d) -> p h d", h=HPC),
                V_SCALE, None, ALU.mult)

        def emit_proj_single(mt, tc8, eng="dve", dmaq="sync", pool=None):
            sl = slice(tc8 * TCH, (tc8 + 1) * TCH)
            po = (pool or psA).tile([128, TCH], F32,
                                    tag="a" if pool is None else "st",
                                    name=f"po{mt}_{tc8}")
            lsl = slice((tc8 % 2) * TCH, (tc8 % 2 + 1) * TCH)
            for cht in range(2):
                nc.tensor.matmul(po[:],
                                 wp_sb[:, cht, mt * 128:(mt + 1) * 128],
                                 yn[(tc8 // 2, cht)][:, lsl],
                                 start=(cht == 0), stop=(cht == 1))
            ot = outp.tile([128, TCH], BF16, tag="ots")
            if eng == "act":
                nc.scalar.activation(ot[:], po[:], AF.Copy, scale=PROJ_SCALE)
            elif eng == "pool":
                nc.gpsimd.tensor_scalar(ot[:], po[:], PROJ_SCALE, None,
                                        ALU.mult)
            else:
                nc.vector.tensor_scalar(ot[:], po[:], PROJ_SCALE, None,
                                        ALU.mult)
            getattr(nc, dmaq).dma_start(outT[mt * 128:(mt + 1) * 128, sl],
                                        ot[:])

        def emit_proj_pair(mt, pair, engs=("dve", "dve"), dmaq="gpsimd"):
            ot = outp.tile([128, 2, TCH], BF16, tag="ot")
            for half in range(2):
                tc8 = pair * 2 + half
                sl = slice(tc8 * TCH, (tc8 + 1) * TCH)
                po = psA.tile([128, TCH], F32, tag="a", name=f"po{mt}_{tc8}")
                lsl = slice((tc8 % 2) * TCH, (tc8 % 2 + 1) * TCH)
                for cht in range(2):
                    nc.tensor.matmul(po[:],
                                     wp_sb[:, cht, mt * 128:(mt + 1) * 128],
                                     yn[(tc8 // 2, cht)][:, lsl],
                                     start=(cht == 0), stop=(cht == 1))
                if engs[half] == "act":
                    nc.scalar.activation(ot[:, half], po[:], AF.Copy,
                                         scale=PROJ_SCALE)
                elif engs[half] == "pool":
                    nc.gpsimd.tensor_scalar(ot[:, half], po[:], PROJ_SCALE,
                                            None, ALU.mult)
                else:
                    nc.vector.tensor_scalar(ot[:, half], po[:], PROJ_SCALE,
                                            None, ALU.mult)
            getattr(nc, dmaq).dma_start(
                outT[mt * 128:(mt + 1) * 128,
                     pair * 2 * TCH:(pair * 2 + 2) * TCH], ot[:])

        fillers: list = []

        def drain(n):
            # qkv fillers gate future exps: keep them at normal priority.
            # proj fillers are pure sinks: push them to low priority.
            save = tc.cur_priority
            try:
                for _ in range(min(n, len(fillers))):
                    kind, fn = fillers.pop(0)
                    tc.cur_priority = save + {"gate": 8000, "v": 12000,
                                              "sink": 16000}[kind]
                    fn()
            finally:
                tc.cur_priority = save
            return

        ys_tiles: dict = {}

        def emit_attn_head(j2, h, fill_every=2, fill_at=None,
                           split_exp=False):
            kmt = h // 2
            qmt = h // 2
            nkt = 8 * j2 + 8
            q0 = j2 * QW
            # yp split per PSUM bank: unit N+1's bank-A AVs then only WAR
            # against unit N's bank-A reads (done mid-unit), not its final
            # bank-B normalize (pool reuse deps are tile-granular).
            ypA = psY.tile([128, 4, 128], F32, tag="ypA", name=f"ypA{j2}_{h}")
            ypB = psY.tile([128, 4, 128], F32, tag="ypB", name=f"ypB{j2}_{h}")
            yp = lambda j: (ypA if j < 4 else ypB)[:, j % 4]
            rc = rcp.tile([128, 8], F32, tag="rc", name=f"rc{j2}_{h}")
            p0 = (h % 2) * 64
            if h % 2 == 0:
                ysc = ysp.tile([128, 8, 128], BF16, tag="ys",
                               name=f"ys{j2}_{h // 2}")
                ys_tiles[(j2, h // 2)] = ysc
            else:
                ysc = ys_tiles[(j2, h // 2)]
            ys = ysc[:, :, p0:p0 + HD]
            for kt in range(nkt):
                lead = (kt // 8 == j2)
                cs = 128 * (kt % 8) if lead else 0
                kp = (h % 2) * 64
                k_lhs = khl[kp:kp + 64, kmt, :, kt * 128:(kt + 1) * 128]
                st = psS.tile([128, QW], F32, tag="st", name=f"st{j2}_{h}_{kt}")
                ranges = ((cs, 512), (512, QW)) if cs < 512 else ((cs, QW),)
                for lo, hi in ranges:
                    # each range opens its own PSUM bank group (start=True);
                    # a start=False matmul here would accumulate onto stale
                    # bank contents from the previous st tile use.
                    q_rhs = q8[kp:kp + 64, qmt, q0 + lo:q0 + hi] \
                        .unsqueeze(1).broadcast_to([64, 2, hi - lo])
                    nc.tensor.matmul(st[:, lo:hi], k_lhs, q_rhs,
                                     start=True, stop=True, perf_mode=DR)
                if lead:
                    # causal mask: st[:, diag] += A^T B (-16384 above diag)
                    nc.tensor.matmul(st[:, cs:cs + 128], mask_sb[:, 0, :],
                                     mask_sb[:, 1, :], start=False, stop=True,
                                     skip_group_check=True)
                pt = ptp.tile([128, QW], BF16, tag="pt")
                if split_exp and cs < 512:
                    # halve the first unit's exps so the stream starts as
                    # soon as the first q8 chunk lands
                    nc.scalar.activation(pt[:, cs:512], st[:, cs:512],
                                         AF.Exp, scale=EXP_SCALE)
                    nc.scalar.activation(pt[:, 512:], st[:, 512:], AF.Exp,
                                         scale=EXP_SCALE)
                else:
                    nc.scalar.activation(pt[:, cs:], st[:, cs:], AF.Exp,
                                         scale=EXP_SCALE)
                # PSUM zero regions are bank-wide (2KB): only one accumulation
                # group per bank. Open each bank once (j=0/j=4 at kt=0); the
                # bank-wide pending-zero gives the other subtiles their
                # initial zeroing; close with the bank's last accumulation.
                j0 = max(0, kt - 8 * j2)
                for j in range(j0, 8):
                    nc.tensor.matmul(yp(j)[:, 0:HD + 1],
                                     pt[:, j * 128:(j + 1) * 128],
                                     v1[:, h, kt, :],
                                     start=(kt == 0 and j % 4 == 0),
                                     stop=(j % 4 == 3 and kt == 8 * j2 + j))
                if kt == 8 * j2 + 3:
                    # bank 0 (subtiles 0-3) just closed: normalize its half
                    # now, 4 k-tiles before the unit ends
                    nc.vector.reciprocal(rc[:, 0:4], ypA[:, :, HD])
                    for j in range(4):
                        nc.vector.tensor_scalar(ys[:, j, :], yp(j)[:, 0:HD],
                                                rc[:, j:j + 1], None, ALU.mult)
                if fill_at is not None:
                    if kt in fill_at:
                        drain(1)
                elif (kt + 1) % fill_every == 0:
                    drain(1)
            # bank 1 half (the last head's copies split across DVE/ACT to
            # shorten the post-stream tail)
            nc.vector.reciprocal(rc[:, 4:8], ypB[:, :, HD])
            tail_head = (j2 == 1 and h == HPC - 1)
            for j in range(4, 8):
                if tail_head and j % 2 == 1:
                    nc.scalar.activation(ys[:, j, :], yp(j)[:, 0:HD],
                                         AF.Copy, scale=rc[:, j:j + 1])
                else:
                    nc.vector.tensor_scalar(ys[:, j, :], yp(j)[:, 0:HD],
                                            rc[:, j:j + 1], None, ALU.mult)
            if _DEBUG and h == 0:
                nc.sync.dma_start(t_in["ys_dbg"][:, j2], ys[:])
                nc.sync.dma_start(t_in["rc_dbg"][:, j2], rc[:])

        def emit_dphase_half(j2, cht, half, ysc, eng="dve"):
            # transpose back: yn[ch, tok] = ys[q, ch].T
            # mid-stream phases use the XBAR DMA transpose on the idle SP
            # queue (frees PE + DVE); eng="pe" keeps the identity-matmul path
            # for the latency-critical final phase.
            if eng == "dma":
                for jj in range(4):
                    j = half * 4 + jj
                    t0 = j * 128
                    nc.sync.dma_start(yn[(j2, cht)][:, t0:t0 + 128],
                                      ysc[:, j, :], transpose=True)
                return
            ys_pair = [ysc[:, :, 0:HD], ysc[:, :, HD:2 * HD]]
            if True:
                dout = psA.tile([128, 512], F32, tag="a",
                                name=f"do{j2}_{cht}_{half}")
                for hh in range(2):
                    for jj in range(4):
                        j = half * 4 + jj
                        nc.tensor.matmul(dout[hh * 64:(hh + 1) * 64,
                                              jj * 128:(jj + 1) * 128],
                                         ys_pair[hh][:, j, :],
                                         mask_sb[:, 1, :],
                                         start=True, stop=True)
                t0 = half * 512
                ynt = yn[(j2, cht)]
                if eng == "act":
                    nc.scalar.activation(ynt[:, t0:t0 + 512], dout[:],
                                         AF.Copy, scale=1.0 / MASK_VAL)
                elif eng == "split":
                    nc.vector.tensor_scalar(ynt[:, t0:t0 + 256],
                                            dout[:, 0:256], 1.0 / MASK_VAL,
                                            None, ALU.mult)
                    nc.scalar.activation(ynt[:, t0 + 256:t0 + 512],
                                         dout[:, 256:512], AF.Copy,
                                         scale=1.0 / MASK_VAL)
                else:
                    nc.vector.tensor_scalar(ynt[:, t0:t0 + 512],
                                            dout[:], 1.0 / MASK_VAL, None,
                                            ALU.mult)

        def emit_dphase(j2, cht, engs=("dve", "dve")):
            ysc = ys_tiles.pop((j2, cht))
            for half in range(2):
                emit_dphase_half(j2, cht, half, ysc, engs[half])

        # ---------- schedule ----------
        _mark(nc, "qkv0")
        for tc8 in range(2):
            for mt in (2, 0):   # k first: the k hi+lo copies gate the
                emit_qk_chunk(tc8, mt, eng="act")  # first S matmul
        save_p = tc.cur_priority
        tc.cur_priority = save_p + 12000
        for kt in range(4):
            emit_v_chunk(kt)
        tc.cur_priority = save_p

        if "attn" not in _ABLATE:
            # All four j2=0 units run early so dphase(0,*) completes and the
            # 16 proj chunks for tokens 0:1024 become PE filler for the
            # three final j2=1 units; fillers sized per unit.
            def qkf(tc8, mt):
                fillers.append(("gate", lambda: emit_qk_chunk(tc8, mt)))

            def vf(kt):
                fillers.append(("v", lambda: emit_v_chunk(kt)))

            _mark(nc, "attn0")
            xload_tail()
            qkf(2, 0); qkf(3, 0)
            for kt in range(4, 8):
                vf(kt)
            qkf(0, 1); qkf(0, 3)
            with tc.high_priority(offset=40000):
                emit_attn_head(0, 0, fill_at=set(range(8)), split_exp=True)
            qkf(2, 2); qkf(3, 2); qkf(1, 1); qkf(1, 3)
            with tc.high_priority(offset=40000):
                emit_attn_head(0, 1, fill_at={0, 1, 2, 3}, split_exp=True)
            emit_dphase(0, 0, engs=("dve", "dve"))
            for kt in range(8, 16):
                vf(kt)
            with tc.high_priority(offset=40000):
                emit_attn_head(1, 0, fill_at=set(range(8)))
            qkf(2, 1); qkf(3, 1); qkf(2, 3); qkf(3, 3)
            with tc.high_priority(offset=40000):
                emit_attn_head(0, 2, fill_at={1, 3, 5, 7})
            with tc.high_priority(offset=40000):
                emit_attn_head(0, 3, fill_at={1, 3, 5, 7})
            emit_dphase(0, 1, engs=("dve", "dve"))
            _mark(nc, "attn1")
            for mt in range(NCT):
                fillers.append(("sink", lambda mt=mt: emit_proj_pair(mt, 0)))
            with tc.high_priority(offset=40000):
                emit_attn_head(1, 1, fill_at={3, 7, 11, 15})
            emit_dphase(1, 0, engs=("dma", "dma"))
            with tc.high_priority(offset=40000):
                emit_attn_head(1, 2, fill_at={1, 5, 9, 13})
            with tc.high_priority(offset=40000):
                emit_attn_head(1, 3, fill_at={1, 5, 9, 13})
            _mark(nc, "dphase1")
            drain(len(fillers))
            ysc_t = ys_tiles.pop((1, 1))
            with tc.high_priority(offset=40000):
                emit_dphase_half(1, 1, 0, ysc_t, "dve")
            # tail proj pinned late via wait_until: the scheduler's internal
            # sim underestimates when yn(1,*) lands, and an early slot in
            # the in-order PE queue head-blocks the remaining attention
            # stream behind a not-yet-ready proj matmul.
            wu = float(__import__("os").environ.get("K_TAIL_WU", "0.087"))
            wu2 = float(__import__("os").environ.get("K_TAIL_WU2", "0.004"))
            with tc.tile_wait_until(wu):
                for i, mt in enumerate(range(NCT)):
                    emit_proj_single(mt, 2, eng=("act", "dve")[i % 2],
                                     dmaq=("sync", "gpsimd")[i % 2],
                                     pool=(None, psS)[i % 2])
            with tc.high_priority(offset=40000):
                emit_dphase_half(1, 1, 1, ysc_t, "split")
            with tc.tile_wait_until(wu + wu2):
                for i, mt in enumerate(range(NCT)):
                    emit_proj_single(mt, 3, eng=("dve", "act")[i % 2],
                                     dmaq=("gpsimd", "sync")[i % 2] if i < 6
                                     else ("scalar", "sync")[i % 2],
                                     pool=(None, psS)[i % 2])
        else:
            for tc8 in range(2):
                for mt in (1, 3):
                    emit_qk_chunk(tc8, mt)
            for tc8 in range(2, 4):
                for mt in range(NMT):
                    emit_qk_chunk(tc8, mt)
            for kt in range(8, 16):
                emit_v_chunk(kt)
            for mt in range(NCT):
                emit_proj_pair(mt, 0)

        _mark(nc, "projtail")
        if "proj" not in _ABLATE and "attn" in _ABLATE:
            engs = [("dve", "act"), ("act", "dve")]
            for mt in range(NCT):
                emit_proj_pair(mt, 1, engs=engs[mt % 2], dmaq="sync")

        if _DEBUG:
            nc.sync.dma_start(t_in["q8_dbg"][:], q8[:])
            nc.sync.dma_start(t_in["khl_dbg"][:], khl[:])
            nc.sync.dma_start(t_in["v1_dbg"][:], v1[:])
            for j2 in range(2):
                for cht in range(2):
                    nc.sync.dma_start(
                        t_in["yn_dbg"][:, cht, j2 * QW:(j2 + 1) * QW],
                        yn[(j2, cht)][:])


def _declare_io(nc):
    t_in = {
        # [128, tc8, hl, ct, tok] fp8 (hi/lo planes contiguous per chunk)
        "xq": nc.dram_tensor("xq", [128, NTC, 2, NCT, TCH], FP8,
                             kind="ExternalInput"),
        "w_q0": nc.dram_tensor("w_q0", [128, NCT, 2, 128], FP8,
                               kind="ExternalInput"),
        "w_q1": nc.dram_tensor("w_q1", [128, NCT, 2, 128], FP8,
                               kind="ExternalInput"),
        "w_k0": nc.dram_tensor("w_k0", [128, NCT, 2, 128], FP8,
                               kind="ExternalInput"),
        "w_k1": nc.dram_tensor("w_k1", [128, NCT, 2, 128], FP8,
                               kind="ExternalInput"),
        "w_v": nc.dram_tensor("w_v", [128, NCT, 2, 256], FP8,
                              kind="ExternalInput"),
        "wp_eff": nc.dram_tensor("wp_eff", [128, 2, C], BF16,
                                 kind="ExternalInput"),
        "masks": nc.dram_tensor("masks", [128, 2, 128], BF16,
                                kind="ExternalInput"),
    }
    outT = nc.dram_tensor("outT", [C, T], BF16, kind="ExternalOutput")
    if _DEBUG:
        t_in["q8_dbg"] = nc.dram_tensor("q8_dbg", [128, 2, T], FP8,
                                        kind="ExternalOutput")
        t_in["khl_dbg"] = nc.dram_tensor("khl_dbg", [128, 2, 2, T], FP8,
                                         kind="ExternalOutput")
        t_in["v1_dbg"] = nc.dram_tensor("v1_dbg", [128, HPC, KT, HD + 1],
                                        BF16, kind="ExternalOutput")
        t_in["yn_dbg"] = nc.dram_tensor("yn_dbg", [128, 2, T], BF16,
                                        kind="ExternalOutput")
        t_in["ys_dbg"] = nc.dram_tensor("ys_dbg", [128, 2, 8, HD], BF16,
                                        kind="ExternalOutput")
        t_in["rc_dbg"] = nc.dram_tensor("rc_dbg", [128, 2, 8], F32,
                                        kind="ExternalOutput")
    return t_in, outT


def _build(reps: int = 1):
    nc = bacc.Bacc("TRN2", target_bir_lowering=False, debug=False)
    t_in, outT = _declare_io(nc)
    with tile.TileContext(nc) as tc:
        with ExitStack() as ctx:
            _emit(ctx, tc, t_in, outT, reps=reps)
    nc.compile()
    return nc


def _fp8_split(a: np.ndarray):
    """Return (hi, lo) fp8e4m3 pair with hi + lo ~= a."""
    f8 = ml_dtypes.float8_e4m3
    hi = a.astype(f8)
    lo = (a - hi.astype(np.float32)).astype(f8)
    return hi, lo


def _make_in_maps(inputs: dict) -> list:
    f32 = np.float32
    f8 = ml_dtypes.float8_e4m3
    x = np.asarray(inputs["x"], f32)                     # [B, T, C]
    w_attn = np.asarray(inputs["w_attn"], f32)
    la_attn = np.ascontiguousarray(np.asarray(inputs["la_attn"], f32))
    lb_attn = np.asarray(inputs["lb_attn"], f32)
    w_proj = np.asarray(inputs["w_proj"], f32)
    la_proj = np.asarray(inputs["la_proj"], f32)
    lb_proj = np.asarray(inputs["lb_proj"], f32)

    # fold LoRA into effective weights on the host (input preprocessing)
    Wq = w_attn + 0.5 * lb_attn @ la_attn                # [3C, C]
    Wp = w_proj + 0.5 * lb_proj @ la_proj                # [C, C]

    # x: [128, tc8, hl, ct, tok] fp8, scaled by XS
    xq_b = []
    for b in range(B):
        xT = np.ascontiguousarray(x[b].T) * XS           # [C, T]
        xr = xT.reshape(NCT, 128, T)                     # [ct, p, t]
        hi, lo = _fp8_split(xr)
        arr = np.empty((128, NTC, 2, NCT, TCH), f8)
        for tc8 in range(NTC):
            sl = slice(tc8 * TCH, (tc8 + 1) * TCH)
            arr[:, tc8, 0] = hi[:, :, sl].transpose(1, 0, 2)
            arr[:, tc8, 1] = lo[:, :, sl].transpose(1, 0, 2)
        xq_b.append(arr)

    k_idx = np.arange(128)[:, None]
    q_idx = np.arange(128)[None, :]
    masks = np.zeros((128, 2, 128), ml_dtypes.bfloat16)
    masks[:, 0, :] = (q_idx > k_idx)         # A[j, k] = 1 iff k > j
    masks[:, 1, :] = MASK_VAL * (q_idx == k_idx)

    in_maps = []
    for core in range(NCORES):
        b, g = core // 4, core % 4
        ch0 = g * CH
        rows = np.r_[ch0:ch0 + CH, C + ch0:C + ch0 + CH,
                     2 * C + ch0:2 * C + ch0 + CH]
        # [p, ct, hl, r] = fp8 split of (WS * Wq).T[ct*128+p, r], per row block
        wq_s = np.ascontiguousarray(
            (WS * Wq[rows]).T.reshape(NCT, 128, NQR).transpose(1, 0, 2))
        w_hi, w_lo = _fp8_split(wq_s)                    # [p, ct, 768]
        whl = np.stack([w_hi, w_lo], axis=2)             # [p, ct, 2, 768]
        # [p, cht, c] = Wp.T[ch0+cht*128+p, c]
        wp_eff = np.ascontiguousarray(
            Wp[:, ch0:ch0 + CH].T.reshape(2, 128, C).transpose(1, 0, 2)
        ).astype(ml_dtypes.bfloat16)
        in_maps.append({
            "xq": xq_b[b],
            "w_q0": np.ascontiguousarray(whl[:, :, :, 0:128]),
            "w_q1": np.ascontiguousarray(whl[:, :, :, 128:256]),
            "w_k0": np.ascontiguousarray(whl[:, :, :, 256:384]),
            "w_k1": np.ascontiguousarray(whl[:, :, :, 384:512]),
            "w_v": np.ascontiguousarray(whl[:, :, :, 512:768]),
            "wp_eff": wp_eff,
            "masks": masks,
        })
    return in_maps


def _execute(inputs: dict, trace: bool = False):
    if "nc" not in _CACHE:
        _CACHE["nc"] = _build()
    nc = _CACHE["nc"]
    in_maps = _make_in_maps(inputs)
    res = run_bass_kernel_spmd(nc, in_maps, core_ids=list(range(NCORES)),
                               trace=trace)
    out = np.empty((B, T, C), np.float32)
    for b in range(B):
        acc = np.zeros((C, T), np.float32)
        for g in range(4):
            acc += np.asarray(res.results[b * 4 + g]["outT"], dtype=np.float32)
        out[b] = acc.T
    return out, res


def kernel(**inputs) -> np.ndarray:
    out, _ = _execute(inputs, trace=False)
    return out
